# revision 20
# baseline (speedup 1.0000x reference)
"""ContentGuidedAttention Trainium2 kernel.

Full NxN single-head cross-attention + out-proj + residual + LayerNorm,
for B=4, C=256, H=W=64 (N=4096 tokens), distributed over 8 NeuronCores:
core i handles batch i//2, query-half i%2 (2048 queries, all 4096 keys).
No collectives: K/V are computed redundantly on the two cores sharing a
batch (~5% extra FLOPs).

Layout strategy (all channel-major, zero transposes):
  - Q^T, K^T computed as [C, n] (channels on partitions) in bf16
  - V computed token-major [n, C] in bf16
  - S^T = K Q^T computed as [k, q] psum tiles; exp on ACT -> P^T bf16
  - softmax denominator: elementwise chunk-tree (GPSIMD+DVE) then a
    ones-vector matmul reduces 128 partitions -> [1, q]
  - PV: O^T[c, q] = sum_k V[k,c] P^T[k,q]; out-proj keeps channel-major
  - LN stats via ones-matmuls ([1,q] rows), row->all-partition
    replication via one-hot selector matmuls, affine via ACT Identity
    with per-partition scale/bias.
Projection matmuls run in float32r (full PE rate at free dim >= 256).
"""

import numpy as np

import concourse.bass as bass
import concourse.mybir as mybir
import concourse.tile as tile
from concourse import bacc
from concourse.bass import ds, ts
from concourse.bass_utils import run_bass_kernel_spmd

F32 = mybir.dt.float32
F32R = mybir.dt.float32r
BF16 = mybir.dt.bfloat16
AF = mybir.ActivationFunctionType
OP = mybir.AluOpType

B = 4
C = 256
N = 4096          # tokens per batch
NQ = 2048         # queries per core
QB = 512          # query block
NQB = NQ // QB    # 4
NKC = N // 128    # 32 key chunks
SCALE = (C // 8) ** -0.5
LN_EPS = 1e-5

_CACHE = {}


def _build_nc(dbg=False):
    nc = bacc.Bacc("TRN2", target_bir_lowering=False, debug=False)

    low_d = nc.declare_dram_parameter("low", [C, NQ], F32R, isOutput=False)
    high_d = nc.declare_dram_parameter("high", [C, N], F32R, isOutput=False)
    # weights are passed pre-transposed: [c_in, c_out]
    wq_d = nc.declare_dram_parameter("wq", [C, C], F32R, isOutput=False)
    wk_d = nc.declare_dram_parameter("wk", [C, C], F32R, isOutput=False)
    wv_d = nc.declare_dram_parameter("wv", [C, C], F32R, isOutput=False)
    wo_d = nc.declare_dram_parameter("wo", [C, C], F32R, isOutput=False)
    qb_d = nc.declare_dram_parameter("qb", [C], F32, isOutput=False)
    kb_d = nc.declare_dram_parameter("kb", [C], F32, isOutput=False)
    vb_d = nc.declare_dram_parameter("vb", [C], F32R, isOutput=False)
    ob_d = nc.declare_dram_parameter("ob", [C], F32, isOutput=False)
    lng_d = nc.declare_dram_parameter("lng", [C], F32, isOutput=False)
    lnb_d = nc.declare_dram_parameter("lnb", [C], F32, isOutput=False)
    out_d = nc.declare_dram_parameter("out", [C, NQ], F32, isOutput=True)
    dbg_d = {}
    if dbg:
        for nm, shp, dt_ in [
            ("dbg_rcp", [128, 512], F32), ("dbg_mu", [128, 512], F32),
            ("dbg_ms", [128, 512], F32), ("dbg_rstd", [128, 512], F32),
            ("dbg_tT", [128, 512], BF16), ("dbg_ot", [128, 2, QB], F32),
            ("dbg_qt", [128, 2, QB], BF16), ("dbg_kt", [128, 2, 512], BF16),
            ("dbg_v", [128, 4, C], BF16), ("dbg_pt", [128, 8, QB], BF16),
        ]:
            dbg_d[nm] = nc.declare_dram_parameter(nm, shp, dt_, isOutput=True)

    with tile.TileContext(nc) as tc:
        with (
            tc.tile_pool(name="persist", bufs=1) as pp,
            tc.tile_pool(name="qt", bufs=2) as qt_pool,
            tc.tile_pool(name="high", bufs=2) as high_pool,
            tc.tile_pool(name="pt", bufs=5) as pt_pool,
            tc.tile_pool(name="ot", bufs=2) as ot_pool,
            tc.tile_pool(name="scratch", bufs=2) as scr_pool,
            tc.tile_pool(name="outsb", bufs=2) as out_pool,
            tc.tile_pool(name="st_ps", bufs=2, space="PSUM") as st_ps,
            tc.tile_pool(name="acc_ps", bufs=2, space="PSUM") as acc_ps,
            tc.tile_pool(name="row_ps", bufs=2, space="PSUM") as row_ps,
        ):
            # ---------------- constants / parameters ----------------
            # all four weight matrices in one tile: [cin_p, cin_chunk, w, cout]
            w_sb = pp.tile([128, 2, 4, C], F32R)
            for j in range(2):
                for wi, wd in enumerate([wq_d, wk_d, wv_d, wo_d]):
                    nc.sync.dma_start(
                        out=w_sb[:, j, wi, :], in_=wd[ds(j * 128, 128), :]
                    )

            # per-partition scalars: qb, kb, ob, lng, lnb as [128, 2] each
            pvec = pp.tile([128, 10], F32)
            for vi, vd in enumerate([qb_d, kb_d, ob_d, lng_d, lnb_d]):
                nc.sync.dma_start(
                    out=pvec[:, ds(vi * 2, 2)],
                    in_=vd[:].rearrange("(j p) -> p j", p=128),
                )
            QBIAS, KBIAS, OBIAS, LNG, LNB = 0, 2, 4, 6, 8

            # v bias as a [1, 256] row (free-dim vector)
            vbrow = pp.tile([1, C], F32R)
            nc.sync.dma_start(out=vbrow[:, :], in_=vb_d[:].unsqueeze(0))

            # memset cannot emit float32r; stage in f32 and copy (the
            # DVE tensor_copy performs the f32 -> f32r rounding walrus wants)
            stage = pp.tile([128, 512], F32)
            ones1 = pp.tile([1, 128], F32R)      # K=1 replication lhsT
            nc.vector.memset(stage[ds(0, 1), 0:128], 1.0)
            nc.vector.tensor_copy(ones1[:, :], stage[ds(0, 1), 0:128])
            ones128 = pp.tile([128, 1], F32R)    # partition-reduce lhsT (f32r)
            nc.vector.memset(stage[:, 0:1], 1.0)
            nc.vector.tensor_copy(ones128[:, :], stage[:, 0:1])
            ones128b = pp.tile([128, 1], BF16)  # partition-reduce lhsT (bf16)
            nc.vector.memset(ones128b[:, :], 1.0)
            # one-hot selectors: sel[p, b, m] = (p == 32*b); replicates the
            # row at partition 32*b to all 128 output partitions
            sel = pp.tile([128, 4, 128], F32R)
            nc.vector.memset(stage[:, :], 0.0)
            for p in range(4):
                nc.vector.memset(stage[ds(p * 32, 1), ds(p * 128, 128)], 1.0)
            nc.vector.tensor_copy(
                sel[:, :, :], stage[:, :].rearrange("p (a b) -> p a b", a=4)
            )

            # replicate v bias across partitions: vb_rep[m, c] = vb[c]
            vb_ps = acc_ps.tile([128, C], F32, tag="acc")
            nc.tensor.matmul(
                out=vb_ps[:, :],
                lhsT=ones1[:, :],
                rhs=vbrow[:, :],
                start=True, stop=True,
            )
            vb_rep = pp.tile([128, C], F32)
            nc.vector.tensor_copy(vb_rep[:, :], vb_ps[:, :])

            # ---------------- activations ----------------
            low_sb = pp.tile([128, 2, NQ], F32R)
            for j in range(2):
                nc.sync.dma_start(
                    out=low_sb[:, j, :], in_=low_d[ds(j * 128, 128), :]
                )

            kt_sb = pp.tile([128, 2, N], BF16)    # K^T  [c, k]
            v_sb = pp.tile([128, NKC, C], BF16)   # V    [k, c] token-major

            # K^T and V projections, streaming high in 512-wide key ranges
            for kr in range(N // 512):
                hi = high_pool.tile([128, 2, 512], F32R)
                for j in range(2):
                    nc.sync.dma_start(
                        out=hi[:, j, :],
                        in_=high_d[ds(j * 128, 128), ds(kr * 512, 512)],
                    )
                # K^T: out [cout, k] = sum_cin wk[cin, cout] high[cin, k]
                for c in range(2):
                    kps = st_ps.tile([128, 512], F32, tag="st")
                    for j in range(2):
                        nc.tensor.matmul(
                            out=kps[:, :],
                            lhsT=w_sb[:, j, 1, ds(c * 128, 128)],
                            rhs=hi[:, j, :],
                            start=(j == 0), stop=(j == 1),
                        )
                    nc.vector.tensor_scalar_add(
                        out=kt_sb[:, c, ds(kr * 512, 512)],
                        in0=kps[:, :],
                        scalar1=pvec[:, ds(KBIAS + c, 1)],
                    )
                # V: out [k, cout] = sum_cin high[cin, k] wv[cin, cout]
                for u in range(4):
                    kc = kr * 4 + u
                    vps = st_ps.tile([128, C], F32, tag="st")
                    for j in range(2):
                        nc.tensor.matmul(
                            out=vps[:, :],
                            lhsT=hi[:, j, ds(u * 128, 128)],
                            rhs=w_sb[:, j, 2, :],
                            start=(j == 0), stop=(j == 1),
                        )
                    nc.vector.tensor_add(
                        out=v_sb[:, kc, :], in0=vps[:, :], in1=vb_rep[:, :]
                    )

            # Y = low + o_b + attn_out @ wo^T, kept resident for LN
            y_sb = pp.tile([128, 2, NQ], F32R)
            # stat rows live at partition 32*b (engine start partitions must
            # be 32-aligned); unused partitions stay 1.0 so recip/sqrt of the
            # garbage rows remain finite (they are zeroed by the selectors)
            rows_mu = pp.tile([128, 512], F32)
            nc.vector.memset(rows_mu[:, :], 1.0)
            rows_ms = pp.tile([128, 512], F32)
            nc.vector.memset(rows_ms[:, :], 1.0)
            rcp = pp.tile([128, 512], F32)  # 1/denom row at partition 32*b
            nc.vector.memset(rcp[:, :], 1.0)
            rcp_r = pp.tile([128, 512], F32R)
            nc.vector.memset(stage[:, :], 1.0)
            nc.vector.tensor_copy(rcp_r[:, :], stage[:, :])
            tB = pp.tile([128, 8, 512], BF16)  # tree scratch
            tT = pp.tile([128, 512], BF16)     # tree result

            for b in range(NQB):
                qsl = ds(b * QB, QB)
                # ---- Q^T projection for this block ----
                qt = qt_pool.tile([128, 2, QB], BF16)
                for c in range(2):
                    qps = st_ps.tile([128, QB], F32, tag="st")
                    for j in range(2):
                        nc.tensor.matmul(
                            out=qps[:, :],
                            lhsT=w_sb[:, j, 0, ds(c * 128, 128)],
                            rhs=low_sb[:, j, qsl],
                            start=(j == 0), stop=(j == 1),
                        )
                    nc.vector.tensor_scalar_add(
                        out=qt[:, c, :],
                        in0=qps[:, :],
                        scalar1=pvec[:, ds(QBIAS + c, 1)],
                    )

                # ---- S^T + exp, in pairs of key chunks ----
                quarters = [
                    pt_pool.tile([128, 8, QB], BF16, tag="ptq", name=f"ptq{g}")
                    for g in range(4)
                ]
                for si in range(16):
                    sps = st_ps.tile([128, 2, QB], F32, tag="st")
                    for u in range(2):
                        kc = si * 2 + u
                        for c in range(2):
                            nc.tensor.matmul(
                                out=sps[:, u, :],
                                lhsT=kt_sb[:, c, ds(kc * 128, 128)],
                                rhs=qt[:, c, :],
                                start=(c == 0), stop=(c == 1),
                            )
                    nc.scalar.activation(
                        out=quarters[si // 4][:, ds((si % 4) * 2, 2), :],
                        in_=sps[:, :, :],
                        func=AF.Exp,
                        scale=SCALE,
                    )

                # ---- PV: O^T[c, q] accumulated over 32 key chunks ----
                ot = ot_pool.tile([128, 2, QB], F32R)
                for c in range(2):
                    ops = acc_ps.tile([128, QB], F32, tag="acc")
                    for kc in range(NKC):
                        nc.tensor.matmul(
                            out=ops[:, :],
                            lhsT=v_sb[:, kc, ds(c * 128, 128)],
                            rhs=quarters[kc // 8][:, kc % 8, :],
                            start=(kc == 0), stop=(kc == NKC - 1),
                        )
                    nc.vector.tensor_copy(ot[:, c, :], ops[:, :])

                # ---- softmax denominator: chunk tree + ones matmul ----
                # L1 (gpsimd): quarters 0+1 -> q0, 2+3 -> q2 (in place)
                nc.gpsimd.tensor_add(
                    out=quarters[0][:, :, :],
                    in0=quarters[0][:, :, :], in1=quarters[1][:, :, :],
                )
                nc.gpsimd.tensor_add(
                    out=quarters[2][:, :, :],
                    in0=quarters[2][:, :, :], in1=quarters[3][:, :, :],
                )
                # L2..L5 (DVE)
                nc.vector.tensor_add(
                    out=tB[:, :, :],
                    in0=quarters[0][:, :, :], in1=quarters[2][:, :, :],
                )
                nc.vector.tensor_add(
                    out=tB[:, 0:4, :], in0=tB[:, 0:4, :], in1=tB[:, 4:8, :]
                )
                nc.vector.tensor_add(
                    out=tB[:, 0:2, :], in0=tB[:, 0:2, :], in1=tB[:, 2:4, :]
                )
                nc.vector.tensor_add(
                    out=tT[:, :], in0=tB[:, 0, :], in1=tB[:, 1, :]
                )
                dn_ps = row_ps.tile([1, QB], F32, tag="row")
                nc.tensor.matmul(
                    out=dn_ps[:, :], lhsT=ones128b[:, :], rhs=tT[:, :],
                    start=True, stop=True,
                )
                # custom-DVE ops mis-write at nonzero base partitions on
                # HW: compute at partition 0, then copy to partition 32*b
                rrow = scr_pool.tile([1, 512], F32, tag="rrow")
                nc.vector.reciprocal_approx_fast(out=rrow[:, :], in_=dn_ps[:, :])
                nc.vector.tensor_copy(rcp[ds(b * 32, 1), :], rrow[:, :])
                nc.vector.tensor_copy(rcp_r[ds(b * 32, 1), :], rrow[:, :])
                # replicate 1/denom to all partitions (via SBUF: a DVE
                # tensor_tensor may read at most one PSUM operand)
                rcp_ps = acc_ps.tile([128, QB], F32, tag="acc")
                nc.tensor.matmul(
                    out=rcp_ps[:, :],
                    lhsT=sel[:, b, :].bitcast(F32R),
                    rhs=rcp_r[:, :],
                    start=True, stop=True,
                )
                rcp_rep = scr_pool.tile([128, QB], F32, tag="rcprep")
                nc.vector.tensor_copy(rcp_rep[:, :], rcp_ps[:, :])

                # ---- out-projection + scale + residual + bias ----
                for c in range(2):
                    pps = acc_ps.tile([128, QB], F32, tag="acc")
                    for j in range(2):
                        nc.tensor.matmul(
                            out=pps[:, :],
                            lhsT=w_sb[:, j, 3, ds(c * 128, 128)],
                            rhs=ot[:, j, :],
                            start=(j == 0), stop=(j == 1),
                        )
                    ysc = scr_pool.tile([128, QB], F32, tag="scr")
                    nc.vector.tensor_mul(
                        out=ysc[:, :], in0=pps[:, :], in1=rcp_rep[:, :]
                    )
                    nc.vector.scalar_tensor_tensor(
                        out=y_sb[:, c, qsl],
                        in0=ysc[:, :],
                        scalar=pvec[:, ds(OBIAS + c, 1)],
                        in1=low_sb[:, c, qsl].bitcast(F32),
                        op0=OP.add, op1=OP.add,
                    )

                # ---- LN statistics ----
                sy_ps = row_ps.tile([1, QB], F32, tag="row")
                for c in range(2):
                    nc.tensor.matmul(
                        out=sy_ps[:, :],
                        lhsT=ones128[:, :].bitcast(F32R),
                        rhs=y_sb[:, c, qsl],
                        start=(c == 0), stop=(c == 1),
                    )
                nc.vector.tensor_scalar_mul(
                    out=rows_mu[ds(b * 32, 1), :], in0=sy_ps[:, :], scalar1=1.0 / C
                )
                sy2_ps = row_ps.tile([1, QB], F32, tag="row")
                for c in range(2):
                    ysq = scr_pool.tile([128, QB], F32R, tag="ysq")
                    nc.vector.tensor_mul(
                        out=ysq[:, :],
                        in0=y_sb[:, c, qsl].bitcast(F32),
                        in1=y_sb[:, c, qsl].bitcast(F32),
                    )
                    nc.tensor.matmul(
                        out=sy2_ps[:, :],
                        lhsT=ones128[:, :].bitcast(F32R),
                        rhs=ysq[:, :],
                        start=(c == 0), stop=(c == 1),
                    )
                nc.vector.tensor_scalar_mul(
                    out=rows_ms[ds(b * 32, 1), :], in0=sy2_ps[:, :], scalar1=1.0 / C
                )

            # ---------------- LN epilogue ----------------
            # var = E[y^2] - mu^2 ; rstd = 1/sqrt(var + eps)
            mu2 = pp.tile([128, 512], F32)
            nc.vector.tensor_mul(out=mu2[:, :], in0=rows_mu[:, :], in1=rows_mu[:, :])
            var4 = pp.tile([128, 512], F32)
            nc.vector.tensor_sub(out=var4[:, :], in0=rows_ms[:, :], in1=mu2[:, :])
            epsv = pp.tile([128, 1], F32)
            nc.vector.memset(epsv[:, :], LN_EPS)
            sd4 = pp.tile([128, 512], F32)
            nc.scalar.activation(
                out=sd4[:, :], in_=var4[:, :], func=AF.Sqrt, bias=epsv[:, :]
            )
            rstd4 = pp.tile([128, 512], F32)
            nc.vector.reciprocal_approx_accurate(
                out=rstd4[:, :], in_=sd4[:, :], scratch=mu2[:, :]
            )
            rows_mu_r = pp.tile([128, 512], F32R)
            nc.vector.tensor_copy(rows_mu_r[:, :], rows_mu[:, :])
            rstd4_r = pp.tile([128, 512], F32R)
            nc.vector.tensor_copy(rstd4_r[:, :], rstd4[:, :])

            for b in range(NQB):
                qsl = ds(b * QB, QB)
                mu_ps = acc_ps.tile([128, QB], F32, tag="acc")
                nc.tensor.matmul(
                    out=mu_ps[:, :],
                    lhsT=sel[:, b, :].bitcast(F32R),
                    rhs=rows_mu_r[:, :],
                    start=True, stop=True,
                )
                rs_ps = acc_ps.tile([128, QB], F32, tag="acc")
                nc.tensor.matmul(
                    out=rs_ps[:, :],
                    lhsT=sel[:, b, :].bitcast(F32R),
                    rhs=rstd4_r[:, :],
                    start=True, stop=True,
                )
                for c in range(2):
                    yn = scr_pool.tile([128, QB], F32, tag="scr")
                    nc.vector.tensor_sub(
                        out=yn[:, :],
                        in0=y_sb[:, c, qsl].bitcast(F32),
                        in1=mu_ps[:, :],
                    )
                    nc.vector.tensor_mul(
                        out=yn[:, :], in0=yn[:, :], in1=rs_ps[:, :]
                    )
                    osb = out_pool.tile([128, QB], F32)
                    nc.scalar.activation(
                        out=osb[:, :],
                        in_=yn[:, :],
                        func=AF.Identity,
                        scale=pvec[:, ds(LNG + c, 1)],
                        bias=pvec[:, ds(LNB + c, 1)],
                    )
                    nc.sync.dma_start(
                        out=out_d[ds(c * 128, 128), qsl], in_=osb[:, :]
                    )

            if dbg_d:
                nc.sync.dma_start(out=dbg_d["dbg_rcp"][:, :], in_=rcp[:, :])
                nc.sync.dma_start(out=dbg_d["dbg_mu"][:, :], in_=rows_mu[:, :])
                nc.sync.dma_start(out=dbg_d["dbg_ms"][:, :], in_=rows_ms[:, :])
                nc.sync.dma_start(out=dbg_d["dbg_rstd"][:, :], in_=rstd4[:, :])
                nc.sync.dma_start(out=dbg_d["dbg_tT"][:, :], in_=tT[:, :])
                nc.sync.dma_start(
                    out=dbg_d["dbg_ot"][:, :, :], in_=ot[:, :, :].bitcast(F32)
                )
                nc.sync.dma_start(out=dbg_d["dbg_qt"][:, :, :], in_=qt[:, :, :])
                nc.sync.dma_start(
                    out=dbg_d["dbg_kt"][:, :, :], in_=kt_sb[:, :, 0:512]
                )
                nc.sync.dma_start(out=dbg_d["dbg_v"][:, :, :], in_=v_sb[:, 0:4, :])
                nc.sync.dma_start(
                    out=dbg_d["dbg_pt"][:, :, :], in_=quarters[3][:, :, :]
                )

    nc.compile()
    return nc


def get_nc(dbg=False):
    key = "nc_dbg" if dbg else "nc"
    if key not in _CACHE:
        _CACHE[key] = _build_nc(dbg)
    return _CACHE[key]


def make_in_maps(low, high, q_w, q_b, k_w, k_b, v_w, v_b, o_w, o_b, ln_g, ln_b):
    low_r = np.asarray(low, np.float32).reshape(B, C, N)
    high_r = np.asarray(high, np.float32).reshape(B, C, N)
    f32 = lambda x: np.ascontiguousarray(np.asarray(x, np.float32))
    shared = {
        "wq": f32(np.asarray(q_w, np.float32).T),
        "wk": f32(np.asarray(k_w, np.float32).T),
        "wv": f32(np.asarray(v_w, np.float32).T),
        "wo": f32(np.asarray(o_w, np.float32).T),
        "qb": f32(q_b), "kb": f32(k_b), "vb": f32(v_b), "ob": f32(o_b),
        "lng": f32(ln_g), "lnb": f32(ln_b),
    }
    in_maps = []
    for i in range(8):
        bidx, h = i // 2, i % 2
        in_maps.append({
            "low": f32(low_r[bidx][:, h * NQ:(h + 1) * NQ]),
            "high": f32(high_r[bidx]),
            **shared,
        })
    return in_maps


def assemble(results):
    out = np.empty((B, C, N), np.float32)
    for i in range(8):
        bidx, h = i // 2, i % 2
        out[bidx][:, h * NQ:(h + 1) * NQ] = results[i]["out"]
    return out.reshape(B, C, 64, 64)


def kernel(**inputs) -> np.ndarray:
    nc = get_nc()
    in_maps = make_in_maps(**inputs)
    res = run_bass_kernel_spmd(nc, in_maps, core_ids=list(range(8)))
    return assemble(res.results)


if __name__ == "__main__":
    pass


# revision 21
# speedup vs baseline: 1.0615x; 1.0615x over previous
"""ContentGuidedAttention Trainium2 kernel.

Full NxN single-head cross-attention + out-proj + residual + LayerNorm,
for B=4, C=256, H=W=64 (N=4096 tokens), distributed over 8 NeuronCores:
core i handles batch i//2, query-half i%2 (2048 queries, all 4096 keys).
No collectives: K/V are computed redundantly on the two cores sharing a
batch (~5% extra FLOPs).

Layout strategy (all channel-major, zero transposes):
  - Q^T, K^T computed as [C, n] (channels on partitions) in bf16
  - V computed token-major [n, C] in bf16
  - S^T = K Q^T computed as [k, q] psum tiles; exp on ACT -> P^T bf16
  - softmax denominator: contiguous DVE chunk-tree then a ones-vector
    matmul reduces the 128 partitions -> [1, q]
  - reciprocals and rsqrt run on ACT as exp(-ln x) / exp(-0.5 ln x):
    Ln and Exp share one activation-table set, so no table switches
  - row -> all-partition replication via K=1 ones-column matmuls
  - PV: O^T[c, q] = sum_k V[k,c] P^T[k,q]; out-proj keeps channel-major
  - LN entirely per-query-block, overlapped with the next block's
    attention; affine via ACT Identity with per-partition scale/bias
Projection matmuls run in float32r (full PE rate at free dim >= 256).
"""

import numpy as np

import concourse.bass as bass
import concourse.mybir as mybir
import concourse.tile as tile
from concourse import bacc
from concourse.bass import ds, ts
from concourse.bass_utils import run_bass_kernel_spmd

F32 = mybir.dt.float32
F32R = mybir.dt.float32r
BF16 = mybir.dt.bfloat16
AF = mybir.ActivationFunctionType
OP = mybir.AluOpType

B = 4
C = 256
N = 4096          # tokens per batch
NQ = 2048         # queries per core
QB = 512          # query block
NQB = NQ // QB    # 4
NKC = N // 128    # 32 key chunks
NKR = 4           # key ranges (1024 keys each) for K^T / V tiles
SCALE = (C // 8) ** -0.5
LN_EPS = 1e-5

_CACHE = {}


def _build_nc(dbg=False):
    nc = bacc.Bacc("TRN2", target_bir_lowering=False, debug=False)

    low_d = nc.declare_dram_parameter("low", [C, NQ], F32R, isOutput=False)
    high_d = nc.declare_dram_parameter("high", [C, N], F32R, isOutput=False)
    # weights are passed pre-transposed: [c_in, c_out]
    wq_d = nc.declare_dram_parameter("wq", [C, C], F32R, isOutput=False)
    wk_d = nc.declare_dram_parameter("wk", [C, C], F32R, isOutput=False)
    wv_d = nc.declare_dram_parameter("wv", [C, C], F32R, isOutput=False)
    wo_d = nc.declare_dram_parameter("wo", [C, C], F32R, isOutput=False)
    qb_d = nc.declare_dram_parameter("qb", [C], F32, isOutput=False)
    kb_d = nc.declare_dram_parameter("kb", [C], F32, isOutput=False)
    vb_d = nc.declare_dram_parameter("vb", [C], F32R, isOutput=False)
    ob_d = nc.declare_dram_parameter("ob", [C], F32, isOutput=False)
    lng_d = nc.declare_dram_parameter("lng", [C], F32, isOutput=False)
    lnb_d = nc.declare_dram_parameter("lnb", [C], F32, isOutput=False)
    out_d = nc.declare_dram_parameter("out", [C, NQ], F32, isOutput=True)
    dbg_d = {}
    if dbg:
        for nm, shp, dt_ in [
            ("dbg_rcp", [1, 512], F32), ("dbg_mu", [1, 512], F32),
            ("dbg_var", [1, 512], F32), ("dbg_rstd", [1, 512], F32),
            ("dbg_tT", [128, 512], BF16), ("dbg_ot", [128, 2, QB], F32),
            ("dbg_qt", [128, 2, QB], BF16), ("dbg_kt", [128, 2, 1024], BF16),
            ("dbg_v", [128, 8, C], BF16), ("dbg_pt", [128, 8, QB], BF16),
        ]:
            dbg_d[nm] = nc.declare_dram_parameter(nm, shp, dt_, isOutput=True)

    with tile.TileContext(nc) as tc:
        with (
            tc.tile_pool(name="persist", bufs=1) as pp,
            tc.tile_pool(name="qt", bufs=2) as qt_pool,
            tc.tile_pool(name="high", bufs=2) as high_pool,
            tc.tile_pool(name="pt", bufs=5) as pt_pool,
            tc.tile_pool(name="ot", bufs=2) as ot_pool,
            tc.tile_pool(name="scratch", bufs=2) as scr_pool,
            tc.tile_pool(name="rowscr", bufs=3) as row_pool,
            tc.tile_pool(name="outsb", bufs=2) as out_pool,
            tc.tile_pool(name="st_ps", bufs=2, space="PSUM") as st_ps,
            tc.tile_pool(name="acc_ps", bufs=2, space="PSUM") as acc_ps,
            tc.tile_pool(name="row_ps", bufs=2, space="PSUM") as row_ps,
        ):
            # ---------------- constants / parameters ----------------
            # all four weight matrices in one tile: [cin_p, cin_chunk, w, cout]
            w_sb = pp.tile([128, 2, 4, C], F32R)
            for j in range(2):
                for wi, wd in enumerate([wq_d, wk_d, wv_d, wo_d]):
                    nc.sync.dma_start(
                        out=w_sb[:, j, wi, :], in_=wd[ds(j * 128, 128), :]
                    )

            # v bias as a [1, 256] row (free-dim vector)
            vbrow = pp.tile([1, C], F32R)
            nc.sync.dma_start(out=vbrow[:, :], in_=vb_d[:].unsqueeze(0))

            # memset cannot emit float32r; stage in f32 and copy (the
            # DVE tensor_copy performs the f32 -> f32r rounding walrus wants)
            stage = pp.tile([128, 128], F32)
            ones1 = pp.tile([1, 128], F32R)      # K=1 replication lhsT
            nc.vector.memset(stage[ds(0, 1), :], 1.0)
            nc.vector.tensor_copy(ones1[:, :], stage[ds(0, 1), :])
            ones128 = pp.tile([128, 1], F32R)    # partition-reduce lhsT (f32r)
            nc.vector.memset(stage[:, 0:1], 1.0)
            nc.vector.tensor_copy(ones128[:, :], stage[:, 0:1])
            ones128b = pp.tile([128, 1], BF16)   # partition-reduce lhsT (bf16)
            nc.vector.memset(ones128b[:, :], 1.0)
            epsb = pp.tile([1, 1], F32)          # LN epsilon bias
            nc.vector.memset(epsb[:, :], LN_EPS)

            # per-partition scalars: qb, kb, ob, lng, lnb as [128, 2] each
            pvec = pp.tile([128, 10], F32)
            for vi, vd in enumerate([qb_d, kb_d, ob_d, lng_d, lnb_d]):
                nc.sync.dma_start(
                    out=pvec[:, ds(vi * 2, 2)],
                    in_=vd[:].rearrange("(j p) -> p j", p=128),
                )
            QBIAS, KBIAS, OBIAS, LNG, LNB = 0, 2, 4, 6, 8

            # replicate v bias across partitions: vb_rep[m, c] = vb[c]
            vb_ps = acc_ps.tile([128, C], F32, tag="acc")
            nc.tensor.matmul(
                out=vb_ps[:, :], lhsT=ones1[:, :], rhs=vbrow[:, :],
                start=True, stop=True,
            )
            vb_rep = pp.tile([128, C], F32)
            nc.vector.tensor_copy(vb_rep[:, :], vb_ps[:, :])

            # ---------------- K^T / V projections ----------------
            # per 1024-key-range tiles so attention can start early
            kt_sb = [
                pp.tile([128, 2, 1024], BF16, name=f"kt{r}", tag=f"kt{r}")
                for r in range(NKR)
            ]
            v_sb = [
                pp.tile([128, 8, C], BF16, name=f"v{r}", tag=f"v{r}")
                for r in range(NKR)
            ]
            for kr in range(N // 512):
                hi = high_pool.tile([128, 2, 512], F32R)
                for j in range(2):
                    nc.sync.dma_start(
                        out=hi[:, j, :],
                        in_=high_d[ds(j * 128, 128), ds(kr * 512, 512)],
                    )
                r, h = kr // 2, kr % 2
                # K^T: out [cout, k] = sum_cin wk[cin, cout] high[cin, k]
                for c in range(2):
                    kps = st_ps.tile([128, 512], F32, tag="st")
                    for j in range(2):
                        nc.tensor.matmul(
                            out=kps[:, :],
                            lhsT=w_sb[:, j, 1, ds(c * 128, 128)],
                            rhs=hi[:, j, :],
                            start=(j == 0), stop=(j == 1),
                        )
                    nc.scalar.activation(
                        out=kt_sb[r][:, c, ds(h * 512, 512)],
                        in_=kps[:, :],
                        func=AF.Identity,
                        bias=pvec[:, ds(KBIAS + c, 1)],
                    )
                # V: out [k, cout] = sum_cin high[cin, k] wv[cin, cout]
                for u in range(4):
                    vps = st_ps.tile([128, C], F32, tag="st")
                    for j in range(2):
                        nc.tensor.matmul(
                            out=vps[:, :],
                            lhsT=hi[:, j, ds(u * 128, 128)],
                            rhs=w_sb[:, j, 2, :],
                            start=(j == 0), stop=(j == 1),
                        )
                    nc.vector.tensor_add(
                        out=v_sb[r][:, h * 4 + u, :],
                        in0=vps[:, :], in1=vb_rep[:, :],
                    )

            # ---------------- main loop over query blocks ----------------
            low_sb = pp.tile([128, 2, NQ], F32R)
            for j in range(2):
                nc.sync.dma_start(
                    out=low_sb[:, j, :], in_=low_d[ds(j * 128, 128), :]
                )

            for b in range(NQB):
                qsl = ds(b * QB, QB)
                # ---- Q^T projection for this block ----
                qt = qt_pool.tile([128, 2, QB], BF16)
                for c in range(2):
                    qps = st_ps.tile([128, QB], F32, tag="st")
                    for j in range(2):
                        nc.tensor.matmul(
                            out=qps[:, :],
                            lhsT=w_sb[:, j, 0, ds(c * 128, 128)],
                            rhs=low_sb[:, j, qsl],
                            start=(j == 0), stop=(j == 1),
                        )
                    nc.scalar.activation(
                        out=qt[:, c, :], in_=qps[:, :],
                        func=AF.Identity,
                        bias=pvec[:, ds(QBIAS + c, 1)],
                    )

                # ---- S^T + exp, in pairs of key chunks ----
                quarters = [
                    pt_pool.tile([128, 8, QB], BF16, tag="ptq", name=f"ptq{g}")
                    for g in range(4)
                ]
                for si in range(16):
                    sps = st_ps.tile([128, 2, QB], F32, tag="st")
                    for u in range(2):
                        kc = si * 2 + u
                        for c in range(2):
                            nc.tensor.matmul(
                                out=sps[:, u, :],
                                lhsT=kt_sb[kc // 8][:, c, ds((kc % 8) * 128, 128)],
                                rhs=qt[:, c, :],
                                start=(c == 0), stop=(c == 1),
                            )
                    nc.scalar.activation(
                        out=quarters[si // 4][:, ds((si % 4) * 2, 2), :],
                        in_=sps[:, :, :],
                        func=AF.Exp,
                        scale=SCALE,
                    )

                # ---- PV: O^T[c, q] accumulated over 32 key chunks ----
                ot = ot_pool.tile([128, 2, QB], F32R)
                for c in range(2):
                    ops = acc_ps.tile([128, QB], F32, tag="acc")
                    for kc in range(NKC):
                        nc.tensor.matmul(
                            out=ops[:, :],
                            lhsT=v_sb[kc // 8][:, kc % 8, ds(c * 128, 128)],
                            rhs=quarters[kc // 8][:, kc % 8, :],
                            start=(kc == 0), stop=(kc == NKC - 1),
                        )
                    nc.vector.tensor_copy(ot[:, c, :], ops[:, :])

                # ---- softmax denominator: contiguous DVE chunk tree ----
                fl = [q[:, :, :].rearrange("p a b -> p (a b)") for q in quarters]
                nc.vector.tensor_add(out=fl[0], in0=fl[0], in1=fl[1])
                nc.vector.tensor_add(out=fl[2], in0=fl[2], in1=fl[3])
                nc.vector.tensor_add(out=fl[0], in0=fl[0], in1=fl[2])
                tB = scr_pool.tile([128, 4, QB], BF16, tag="tB")
                nc.vector.tensor_add(
                    out=tB[:, :, :],
                    in0=quarters[0][:, 0:4, :], in1=quarters[0][:, 4:8, :],
                )
                tT2 = scr_pool.tile([128, 2, QB], BF16, tag="tT2")
                nc.vector.tensor_add(
                    out=tT2[:, :, :], in0=tB[:, 0:2, :], in1=tB[:, 2:4, :]
                )
                tT = scr_pool.tile([128, QB], BF16, tag="tT")
                nc.vector.tensor_add(
                    out=tT[:, :], in0=tT2[:, 0, :], in1=tT2[:, 1, :]
                )
                dn_ps = row_ps.tile([1, QB], F32, tag="row")
                nc.tensor.matmul(
                    out=dn_ps[:, :], lhsT=ones128b[:, :], rhs=tT[:, :],
                    start=True, stop=True,
                )
                # 1/denom = exp(-ln(denom)) on ACT (same table set as Exp)
                lnrow = row_pool.tile([1, QB], F32, tag="lnrow")
                nc.scalar.activation(
                    out=lnrow[:, :], in_=dn_ps[:, :], func=AF.Ln
                )
                rcprow = row_pool.tile([1, QB], F32R, tag="rcprow")
                nc.scalar.activation(
                    out=rcprow[:, :], in_=lnrow[:, :], func=AF.Exp, scale=-1.0
                )
                rcp_ps = acc_ps.tile([128, QB], F32, tag="acc")
                nc.tensor.matmul(
                    out=rcp_ps[:, :], lhsT=ones1[:, :], rhs=rcprow[:, :],
                    start=True, stop=True,
                )
                rcp_rep = scr_pool.tile([128, QB], F32, tag="rcprep")
                nc.vector.tensor_copy(rcp_rep[:, :], rcp_ps[:, :])

                # ---- out-projection + 1/denom + residual + out-bias ----
                y_sb = ot_pool.tile([128, 2, QB], F32R, tag="y")
                for c in range(2):
                    pps = acc_ps.tile([128, QB], F32, tag="acc")
                    for j in range(2):
                        nc.tensor.matmul(
                            out=pps[:, :],
                            lhsT=w_sb[:, j, 3, ds(c * 128, 128)],
                            rhs=ot[:, j, :],
                            start=(j == 0), stop=(j == 1),
                        )
                    ysc = scr_pool.tile([128, QB], F32, tag="scr")
                    nc.vector.tensor_mul(
                        out=ysc[:, :], in0=pps[:, :], in1=rcp_rep[:, :]
                    )
                    nc.vector.scalar_tensor_tensor(
                        out=y_sb[:, c, :],
                        in0=ysc[:, :],
                        scalar=pvec[:, ds(OBIAS + c, 1)],
                        in1=low_sb[:, c, qsl].bitcast(F32),
                        op0=OP.add, op1=OP.add,
                    )

                # ---- LN statistics (rows at partition 0) ----
                sy_ps = row_ps.tile([1, QB], F32, tag="row")
                for c in range(2):
                    nc.tensor.matmul(
                        out=sy_ps[:, :],
                        lhsT=ones128[:, :],
                        rhs=y_sb[:, c, :],
                        start=(c == 0), stop=(c == 1),
                    )
                murow = row_pool.tile([1, QB], F32R, tag="murow")
                nc.vector.tensor_scalar_mul(
                    out=murow[:, :], in0=sy_ps[:, :], scalar1=1.0 / C
                )
                sy2_ps = row_ps.tile([1, QB], F32, tag="row")
                for c in range(2):
                    ysq = scr_pool.tile([128, QB], F32R, tag="ysq")
                    nc.vector.tensor_mul(
                        out=ysq[:, :],
                        in0=y_sb[:, c, :].bitcast(F32),
                        in1=y_sb[:, c, :].bitcast(F32),
                    )
                    nc.tensor.matmul(
                        out=sy2_ps[:, :],
                        lhsT=ones128[:, :],
                        rhs=ysq[:, :],
                        start=(c == 0), stop=(c == 1),
                    )
                # var = E[y^2] - mu^2 ; rstd = exp(-0.5 ln(var + eps))
                varrow = row_pool.tile([1, QB], F32, tag="varrow")
                nc.vector.tensor_scalar_mul(
                    out=varrow[:, :], in0=sy2_ps[:, :], scalar1=1.0 / C
                )
                mu2row = row_pool.tile([1, QB], F32, tag="mu2row")
                nc.vector.tensor_mul(
                    out=mu2row[:, :],
                    in0=murow[:, :].bitcast(F32), in1=murow[:, :].bitcast(F32),
                )
                nc.vector.tensor_sub(
                    out=varrow[:, :], in0=varrow[:, :], in1=mu2row[:, :]
                )
                lnv = row_pool.tile([1, QB], F32, tag="lnv")
                nc.scalar.activation(
                    out=lnv[:, :], in_=varrow[:, :], func=AF.Ln, bias=epsb[:, :]
                )
                rstdrow = row_pool.tile([1, QB], F32R, tag="rstdrow")
                nc.scalar.activation(
                    out=rstdrow[:, :], in_=lnv[:, :], func=AF.Exp, scale=-0.5
                )
                if dbg_d and b == NQB - 1:
                    nc.sync.dma_start(out=dbg_d["dbg_rcp"][:, :],
                                      in_=rcprow[:, :].bitcast(F32))
                    nc.sync.dma_start(out=dbg_d["dbg_mu"][:, :],
                                      in_=murow[:, :].bitcast(F32))
                    nc.sync.dma_start(out=dbg_d["dbg_var"][:, :],
                                      in_=varrow[:, :])
                    nc.sync.dma_start(out=dbg_d["dbg_rstd"][:, :],
                                      in_=rstdrow[:, :].bitcast(F32))

                # ---- replicate mu / rstd, normalize, affine, store ----
                mu_ps = acc_ps.tile([128, QB], F32, tag="acc")
                nc.tensor.matmul(
                    out=mu_ps[:, :], lhsT=ones1[:, :], rhs=murow[:, :],
                    start=True, stop=True,
                )
                rs_ps = acc_ps.tile([128, QB], F32, tag="acc")
                nc.tensor.matmul(
                    out=rs_ps[:, :], lhsT=ones1[:, :], rhs=rstdrow[:, :],
                    start=True, stop=True,
                )
                for c in range(2):
                    yn = scr_pool.tile([128, QB], F32, tag="scr")
                    nc.vector.tensor_sub(
                        out=yn[:, :],
                        in0=y_sb[:, c, :].bitcast(F32),
                        in1=mu_ps[:, :],
                    )
                    nc.vector.tensor_mul(
                        out=yn[:, :], in0=yn[:, :], in1=rs_ps[:, :]
                    )
                    osb = out_pool.tile([128, QB], F32)
                    nc.scalar.activation(
                        out=osb[:, :], in_=yn[:, :],
                        func=AF.Identity,
                        scale=pvec[:, ds(LNG + c, 1)],
                        bias=pvec[:, ds(LNB + c, 1)],
                    )
                    nc.sync.dma_start(
                        out=out_d[ds(c * 128, 128), qsl], in_=osb[:, :]
                    )

                if dbg_d and b == NQB - 1:
                    nc.sync.dma_start(out=dbg_d["dbg_tT"][:, :], in_=tT[:, :])
                    nc.sync.dma_start(
                        out=dbg_d["dbg_ot"][:, :, :], in_=ot[:, :, :].bitcast(F32)
                    )
                    nc.sync.dma_start(out=dbg_d["dbg_qt"][:, :, :], in_=qt[:, :, :])
                    nc.sync.dma_start(
                        out=dbg_d["dbg_kt"][:, :, :], in_=kt_sb[0][:, :, :]
                    )
                    nc.sync.dma_start(
                        out=dbg_d["dbg_v"][:, :, :], in_=v_sb[0][:, :, :]
                    )
                    nc.sync.dma_start(
                        out=dbg_d["dbg_pt"][:, :, :], in_=quarters[3][:, :, :]
                    )

    nc.compile()
    return nc


def get_nc(dbg=False):
    key = "nc_dbg" if dbg else "nc"
    if key not in _CACHE:
        _CACHE[key] = _build_nc(dbg)
    return _CACHE[key]


def make_in_maps(low, high, q_w, q_b, k_w, k_b, v_w, v_b, o_w, o_b, ln_g, ln_b):
    low_r = np.asarray(low, np.float32).reshape(B, C, N)
    high_r = np.asarray(high, np.float32).reshape(B, C, N)
    f32 = lambda x: np.ascontiguousarray(np.asarray(x, np.float32))
    shared = {
        "wq": f32(np.asarray(q_w, np.float32).T),
        "wk": f32(np.asarray(k_w, np.float32).T),
        "wv": f32(np.asarray(v_w, np.float32).T),
        "wo": f32(np.asarray(o_w, np.float32).T),
        "qb": f32(q_b), "kb": f32(k_b), "vb": f32(v_b), "ob": f32(o_b),
        "lng": f32(ln_g), "lnb": f32(ln_b),
    }
    in_maps = []
    for i in range(8):
        bidx, h = i // 2, i % 2
        in_maps.append({
            "low": f32(low_r[bidx][:, h * NQ:(h + 1) * NQ]),
            "high": f32(high_r[bidx]),
            **shared,
        })
    return in_maps


def assemble(results):
    out = np.empty((B, C, N), np.float32)
    for i in range(8):
        bidx, h = i // 2, i % 2
        out[bidx][:, h * NQ:(h + 1) * NQ] = results[i]["out"]
    return out.reshape(B, C, 64, 64)


def kernel(**inputs) -> np.ndarray:
    nc = get_nc()
    in_maps = make_in_maps(**inputs)
    res = run_bass_kernel_spmd(nc, in_maps, core_ids=list(range(8)))
    return assemble(res.results)


if __name__ == "__main__":
    pass


# revision 23
# speedup vs baseline: 1.1240x; 1.0588x over previous
"""ContentGuidedAttention Trainium2 kernel.

Full NxN single-head cross-attention + out-proj + residual + LayerNorm,
for B=4, C=256, H=W=64 (N=4096 tokens), distributed over 8 NeuronCores:
core i handles batch i//2, query-half i%2 (2048 queries, all 4096 keys).
No collectives: K/V are computed redundantly on the two cores sharing a
batch (~5% extra FLOPs).

Layout strategy (all channel-major, zero transposes):
  - Q^T, K^T computed as [C, n] (channels on partitions) in bf16
  - V computed token-major [n, C] in bf16
  - S^T = K Q^T computed as [k, q] psum tiles; exp on ACT -> P^T bf16
  - softmax denominator: contiguous DVE chunk-tree then a ones-vector
    matmul reduces the 128 partitions -> [1, q]
  - reciprocals and rsqrt run on ACT as exp(-ln x) / exp(-0.5 ln x):
    Ln and Exp share one activation-table set, so no table switches
  - row -> all-partition replication via K=1 ones-column matmuls
  - PV: O^T[c, q] = sum_k V[k,c] P^T[k,q]; out-proj keeps channel-major
  - LN entirely per-query-block, overlapped with the next block's
    attention; affine via ACT Identity with per-partition scale/bias
Projection matmuls run in float32r (full PE rate at free dim >= 256).
"""

import numpy as np

import concourse.bass as bass
import concourse.mybir as mybir
import concourse.tile as tile
from concourse import bacc
from concourse.bass import ds, ts
from concourse.bass_utils import run_bass_kernel_spmd

F32 = mybir.dt.float32
F32R = mybir.dt.float32r
BF16 = mybir.dt.bfloat16
AF = mybir.ActivationFunctionType
OP = mybir.AluOpType

B = 4
C = 256
N = 4096          # tokens per batch
NQ = 2048         # queries per core
QB = 512          # query block
NQB = NQ // QB    # 4
NKC = N // 128    # 32 key chunks
NKR = 4           # key ranges (1024 keys each) for K^T / V tiles
SCALE = (C // 8) ** -0.5
LN_EPS = 1e-5

_CACHE = {}


def _build_nc(dbg=False):
    nc = bacc.Bacc("TRN2", target_bir_lowering=False, debug=False)

    low_d = nc.declare_dram_parameter("low", [C, NQ], F32R, isOutput=False)
    high_d = nc.declare_dram_parameter("high", [C, N], F32R, isOutput=False)
    # weights are passed pre-transposed: [c_in, c_out]
    wq_d = nc.declare_dram_parameter("wq", [C, C], F32R, isOutput=False)
    wk_d = nc.declare_dram_parameter("wk", [C, C], F32R, isOutput=False)
    wv_d = nc.declare_dram_parameter("wv", [C, C], F32R, isOutput=False)
    wo_d = nc.declare_dram_parameter("wo", [C, C], F32R, isOutput=False)
    qb_d = nc.declare_dram_parameter("qb", [C], F32, isOutput=False)
    kb_d = nc.declare_dram_parameter("kb", [C], F32, isOutput=False)
    vb_d = nc.declare_dram_parameter("vb", [C], F32R, isOutput=False)
    ob_d = nc.declare_dram_parameter("ob", [C], F32, isOutput=False)
    lng_d = nc.declare_dram_parameter("lng", [C], F32, isOutput=False)
    lnb_d = nc.declare_dram_parameter("lnb", [C], F32, isOutput=False)
    out_d = nc.declare_dram_parameter("out", [C, NQ], F32, isOutput=True)
    dbg_d = {}
    if dbg:
        for nm, shp, dt_ in [
            ("dbg_rcp", [1, 512], F32), ("dbg_mu", [1, 512], F32),
            ("dbg_var", [1, 512], F32), ("dbg_rstd", [1, 512], F32),
            ("dbg_tT", [128, 512], BF16), ("dbg_ot", [128, 2, QB], F32),
            ("dbg_qt", [128, 2, QB], BF16), ("dbg_kt", [128, 2, 1024], BF16),
            ("dbg_v", [128, 8, C], BF16), ("dbg_pt", [128, 8, QB], BF16),
        ]:
            dbg_d[nm] = nc.declare_dram_parameter(nm, shp, dt_, isOutput=True)

    with tile.TileContext(nc) as tc:
        with (
            tc.tile_pool(name="persist", bufs=1) as pp,
            tc.tile_pool(name="qt", bufs=2) as qt_pool,
            tc.tile_pool(name="high", bufs=3) as high_pool,
            tc.tile_pool(name="pt", bufs=5) as pt_pool,
            tc.tile_pool(name="ot", bufs=2) as ot_pool,
            tc.tile_pool(name="scratch", bufs=2) as scr_pool,
            tc.tile_pool(name="rowscr", bufs=1) as row_pool,
            tc.tile_pool(name="outsb", bufs=2) as out_pool,
            tc.tile_pool(name="st_ps", bufs=2, space="PSUM") as st_ps,
            tc.tile_pool(name="acc_ps", bufs=3, space="PSUM") as acc_ps,
            tc.tile_pool(name="row_ps", bufs=1, space="PSUM") as row_ps,
        ):
            # ---------------- constants / parameters ----------------
            # one tile per weight matrix ([cin_p, cin_chunk, cout]); K/V
            # weights load first so the K/V projections start ASAP
            wk_sb = pp.tile([128, 2, C], F32R)
            wv_sb = pp.tile([128, 2, C], F32R)
            wq_sb = pp.tile([128, 2, C], F32R)
            wo_sb = pp.tile([128, 2, C], F32R)
            for t, d in [(wk_sb, wk_d), (wv_sb, wv_d), (wq_sb, wq_d),
                         (wo_sb, wo_d)]:
                for j in range(2):
                    nc.sync.dma_start(out=t[:, j, :], in_=d[ds(j * 128, 128), :])

            # v bias as a [1, 256] row (free-dim vector)
            vbrow = pp.tile([1, C], F32R)
            nc.sync.dma_start(out=vbrow[:, :], in_=vb_d[:].unsqueeze(0))

            # memset cannot emit float32r; stage in f32 and copy (the
            # DVE tensor_copy performs the f32 -> f32r rounding walrus wants)
            stage = pp.tile([128, 128], F32)
            ones1 = pp.tile([1, 128], F32R)      # K=1 replication lhsT
            nc.vector.memset(stage[ds(0, 1), :], 1.0)
            nc.vector.tensor_copy(ones1[:, :], stage[ds(0, 1), :])
            ones128 = pp.tile([128, 1], F32R)    # partition-reduce lhsT (f32r)
            nc.vector.memset(stage[:, 0:1], 1.0)
            nc.vector.tensor_copy(ones128[:, :], stage[:, 0:1])
            ones128b = pp.tile([128, 1], BF16)   # partition-reduce lhsT (bf16)
            nc.vector.memset(ones128b[:, :], 1.0)
            epsb = pp.tile([1, 1], F32)          # LN epsilon bias
            nc.vector.memset(epsb[:, :], LN_EPS)

            # per-partition scalars: qb, kb, ob, lng, lnb as [128, 2] each
            pvec = pp.tile([128, 10], F32)
            for vi, vd in enumerate([qb_d, kb_d, ob_d, lng_d, lnb_d]):
                nc.sync.dma_start(
                    out=pvec[:, ds(vi * 2, 2)],
                    in_=vd[:].rearrange("(j p) -> p j", p=128),
                )
            QBIAS, KBIAS, OBIAS, LNG, LNB = 0, 2, 4, 6, 8

            # replicate v bias across partitions: vb_rep[m, c] = vb[c]
            vb_ps = acc_ps.tile([128, C], F32, tag="acc")
            nc.tensor.matmul(
                out=vb_ps[:, :], lhsT=ones1[:, :], rhs=vbrow[:, :],
                start=True, stop=True,
            )
            vb_rep = pp.tile([128, C], F32)
            nc.vector.tensor_copy(vb_rep[:, :], vb_ps[:, :])

            # ---------------- K^T / V projections ----------------
            # per 1024-key-range tiles so attention can start early
            kt_sb = [
                pp.tile([128, 2, 1024], BF16, name=f"kt{r}", tag=f"kt{r}")
                for r in range(NKR)
            ]
            v_sb = [
                pp.tile([128, 8, C], BF16, name=f"v{r}", tag=f"v{r}")
                for r in range(NKR)
            ]
            for kr in range(N // 512):
                hi = high_pool.tile([128, 2, 512], F32R)
                for j in range(2):
                    nc.sync.dma_start(
                        out=hi[:, j, :],
                        in_=high_d[ds(j * 128, 128), ds(kr * 512, 512)],
                    )
                r, h = kr // 2, kr % 2
                # K^T: out [cout, k] = sum_cin wk[cin, cout] high[cin, k]
                for c in range(2):
                    kps = st_ps.tile([128, 512], F32, tag="st")
                    for j in range(2):
                        nc.tensor.matmul(
                            out=kps[:, :],
                            lhsT=wk_sb[:, j, ds(c * 128, 128)],
                            rhs=hi[:, j, :],
                            start=(j == 0), stop=(j == 1),
                        )
                    nc.vector.tensor_scalar_add(
                        out=kt_sb[r][:, c, ds(h * 512, 512)],
                        in0=kps[:, :],
                        scalar1=pvec[:, ds(KBIAS + c, 1)],
                    )
                # V: out [k, cout] = sum_cin high[cin, k] wv[cin, cout]
                for u in range(4):
                    vps = st_ps.tile([128, C], F32, tag="st")
                    for j in range(2):
                        nc.tensor.matmul(
                            out=vps[:, :],
                            lhsT=hi[:, j, ds(u * 128, 128)],
                            rhs=wv_sb[:, j, :],
                            start=(j == 0), stop=(j == 1),
                        )
                    nc.vector.tensor_add(
                        out=v_sb[r][:, h * 4 + u, :],
                        in0=vps[:, :], in1=vb_rep[:, :],
                    )

            # ---------------- main loop over query blocks ----------------
            low_sb = pp.tile([128, 2, NQ], F32R)
            for j in range(2):
                nc.sync.dma_start(
                    out=low_sb[:, j, :], in_=low_d[ds(j * 128, 128), :]
                )

            for b in range(NQB):
                qsl = ds(b * QB, QB)
                # ---- Q^T projection for this block ----
                qt = qt_pool.tile([128, 2, QB], BF16)
                for c in range(2):
                    qps = st_ps.tile([128, QB], F32, tag="st")
                    for j in range(2):
                        nc.tensor.matmul(
                            out=qps[:, :],
                            lhsT=wq_sb[:, j, ds(c * 128, 128)],
                            rhs=low_sb[:, j, qsl],
                            start=(j == 0), stop=(j == 1),
                        )
                    nc.vector.tensor_scalar_add(
                        out=qt[:, c, :], in0=qps[:, :],
                        scalar1=pvec[:, ds(QBIAS + c, 1)],
                    )

                # ---- S^T + exp, in pairs of key chunks ----
                quarters = [
                    pt_pool.tile([128, 8, QB], BF16, tag="ptq", name=f"ptq{g}")
                    for g in range(4)
                ]
                for si in range(16):
                    sps = st_ps.tile([128, 2, QB], F32, tag="st")
                    for u in range(2):
                        kc = si * 2 + u
                        for c in range(2):
                            nc.tensor.matmul(
                                out=sps[:, u, :],
                                lhsT=kt_sb[kc // 8][:, c, ds((kc % 8) * 128, 128)],
                                rhs=qt[:, c, :],
                                start=(c == 0), stop=(c == 1),
                            )
                    nc.scalar.activation(
                        out=quarters[si // 4][:, ds((si % 4) * 2, 2), :],
                        in_=sps[:, :, :],
                        func=AF.Exp,
                        scale=SCALE,
                    )

                # ---- PV: O^T[c, q] accumulated over 32 key chunks ----
                ot = ot_pool.tile([128, 2, QB], F32R)
                for c in range(2):
                    ops = acc_ps.tile([128, QB], F32, tag="acc")
                    for kc in range(NKC):
                        nc.tensor.matmul(
                            out=ops[:, :],
                            lhsT=v_sb[kc // 8][:, kc % 8, ds(c * 128, 128)],
                            rhs=quarters[kc // 8][:, kc % 8, :],
                            start=(kc == 0), stop=(kc == NKC - 1),
                        )
                    nc.vector.tensor_copy(ot[:, c, :], ops[:, :])

                # ---- softmax denominator: contiguous DVE chunk tree ----
                fl = [q[:, :, :].rearrange("p a b -> p (a b)") for q in quarters]
                nc.vector.tensor_add(out=fl[0], in0=fl[0], in1=fl[1])
                nc.vector.tensor_add(out=fl[2], in0=fl[2], in1=fl[3])
                nc.vector.tensor_add(out=fl[0], in0=fl[0], in1=fl[2])
                tB = scr_pool.tile([128, 4, QB], BF16, tag="tB")
                nc.vector.tensor_add(
                    out=tB[:, :, :],
                    in0=quarters[0][:, 0:4, :], in1=quarters[0][:, 4:8, :],
                )
                tT2 = scr_pool.tile([128, 2, QB], BF16, tag="tT2")
                nc.vector.tensor_add(
                    out=tT2[:, :, :], in0=tB[:, 0:2, :], in1=tB[:, 2:4, :]
                )
                tT = scr_pool.tile([128, QB], BF16, tag="tT")
                nc.vector.tensor_add(
                    out=tT[:, :], in0=tT2[:, 0, :], in1=tT2[:, 1, :]
                )
                dn_ps = row_ps.tile([1, QB], F32, tag="row")
                nc.tensor.matmul(
                    out=dn_ps[:, :], lhsT=ones128b[:, :], rhs=tT[:, :],
                    start=True, stop=True,
                )
                # 1/denom = exp(-ln(denom)) on ACT (same table set as Exp)
                lnrow = row_pool.tile([1, QB], F32, tag="lnrow")
                nc.scalar.activation(
                    out=lnrow[:, :], in_=dn_ps[:, :], func=AF.Ln
                )
                rcprow = row_pool.tile([1, QB], F32, tag="rcprow")
                nc.scalar.activation(
                    out=rcprow[:, :], in_=lnrow[:, :], func=AF.Exp, scale=-1.0
                )
                rcp_rep = scr_pool.tile([128, QB], F32, tag="rcprep")
                nc.gpsimd.partition_broadcast(rcp_rep[:, :], rcprow[:, :])

                # ---- out-projection + 1/denom + residual + out-bias ----
                y_sb = ot_pool.tile([128, 2, QB], F32R, tag="y")
                for c in range(2):
                    pps = acc_ps.tile([128, QB], F32, tag="acc")
                    for j in range(2):
                        nc.tensor.matmul(
                            out=pps[:, :],
                            lhsT=wo_sb[:, j, ds(c * 128, 128)],
                            rhs=ot[:, j, :],
                            start=(j == 0), stop=(j == 1),
                        )
                    ysc = scr_pool.tile([128, QB], F32, tag="scr")
                    nc.vector.tensor_mul(
                        out=ysc[:, :], in0=pps[:, :], in1=rcp_rep[:, :]
                    )
                    nc.vector.scalar_tensor_tensor(
                        out=y_sb[:, c, :],
                        in0=ysc[:, :],
                        scalar=pvec[:, ds(OBIAS + c, 1)],
                        in1=low_sb[:, c, qsl].bitcast(F32),
                        op0=OP.add, op1=OP.add,
                    )

                # ---- LN statistics (rows at partition 0) ----
                sy_ps = row_ps.tile([1, QB], F32, tag="row")
                for c in range(2):
                    nc.tensor.matmul(
                        out=sy_ps[:, :],
                        lhsT=ones128[:, :],
                        rhs=y_sb[:, c, :],
                        start=(c == 0), stop=(c == 1),
                    )
                murow = row_pool.tile([1, QB], F32, tag="murow")
                nc.vector.tensor_scalar_mul(
                    out=murow[:, :], in0=sy_ps[:, :], scalar1=1.0 / C
                )
                sy2_ps = row_ps.tile([1, QB], F32, tag="row")
                for c in range(2):
                    ysq = scr_pool.tile([128, QB], F32R, tag="ysq")
                    nc.vector.tensor_mul(
                        out=ysq[:, :],
                        in0=y_sb[:, c, :].bitcast(F32),
                        in1=y_sb[:, c, :].bitcast(F32),
                    )
                    nc.tensor.matmul(
                        out=sy2_ps[:, :],
                        lhsT=ones128[:, :],
                        rhs=ysq[:, :],
                        start=(c == 0), stop=(c == 1),
                    )
                # var = E[y^2] - mu^2 ; rstd = exp(-0.5 ln(var + eps))
                varrow = row_pool.tile([1, QB], F32, tag="varrow")
                nc.vector.tensor_scalar_mul(
                    out=varrow[:, :], in0=sy2_ps[:, :], scalar1=1.0 / C
                )
                mu2row = row_pool.tile([1, QB], F32, tag="mu2row")
                nc.vector.tensor_mul(
                    out=mu2row[:, :],
                    in0=murow[:, :], in1=murow[:, :],
                )
                nc.vector.tensor_sub(
                    out=varrow[:, :], in0=varrow[:, :], in1=mu2row[:, :]
                )
                lnv = row_pool.tile([1, QB], F32, tag="lnv")
                nc.scalar.activation(
                    out=lnv[:, :], in_=varrow[:, :], func=AF.Ln, bias=epsb[:, :]
                )
                rstdrow = row_pool.tile([1, QB], F32, tag="rstdrow")
                nc.scalar.activation(
                    out=rstdrow[:, :], in_=lnv[:, :], func=AF.Exp, scale=-0.5
                )
                if dbg_d and b == NQB - 1:
                    nc.sync.dma_start(out=dbg_d["dbg_rcp"][:, :], in_=rcprow[:, :])
                    nc.sync.dma_start(out=dbg_d["dbg_mu"][:, :], in_=murow[:, :])
                    nc.sync.dma_start(out=dbg_d["dbg_var"][:, :],
                                      in_=varrow[:, :])
                    nc.sync.dma_start(out=dbg_d["dbg_rstd"][:, :], in_=rstdrow[:, :])

                # ---- replicate mu / rstd, normalize, affine, store ----
                mu_rep = scr_pool.tile([128, QB], F32, tag="murep")
                nc.gpsimd.partition_broadcast(mu_rep[:, :], murow[:, :])
                rs_rep = scr_pool.tile([128, QB], F32, tag="rsrep")
                nc.gpsimd.partition_broadcast(rs_rep[:, :], rstdrow[:, :])
                for c in range(2):
                    yn = scr_pool.tile([128, QB], F32, tag="scr")
                    nc.vector.tensor_sub(
                        out=yn[:, :],
                        in0=y_sb[:, c, :].bitcast(F32),
                        in1=mu_rep[:, :],
                    )
                    nc.vector.tensor_mul(
                        out=yn[:, :], in0=yn[:, :], in1=rs_rep[:, :]
                    )
                    osb = out_pool.tile([128, QB], F32)
                    nc.vector.tensor_scalar(
                        out=osb[:, :], in0=yn[:, :],
                        scalar1=pvec[:, ds(LNG + c, 1)],
                        scalar2=pvec[:, ds(LNB + c, 1)],
                        op0=OP.mult, op1=OP.add,
                    )
                    nc.sync.dma_start(
                        out=out_d[ds(c * 128, 128), qsl], in_=osb[:, :]
                    )

                if dbg_d and b == NQB - 1:
                    nc.sync.dma_start(out=dbg_d["dbg_tT"][:, :], in_=tT[:, :])
                    nc.sync.dma_start(
                        out=dbg_d["dbg_ot"][:, :, :], in_=ot[:, :, :].bitcast(F32)
                    )
                    nc.sync.dma_start(out=dbg_d["dbg_qt"][:, :, :], in_=qt[:, :, :])
                    nc.sync.dma_start(
                        out=dbg_d["dbg_kt"][:, :, :], in_=kt_sb[0][:, :, :]
                    )
                    nc.sync.dma_start(
                        out=dbg_d["dbg_v"][:, :, :], in_=v_sb[0][:, :, :]
                    )
                    nc.sync.dma_start(
                        out=dbg_d["dbg_pt"][:, :, :], in_=quarters[3][:, :, :]
                    )

    nc.compile()
    return nc


def get_nc(dbg=False):
    key = "nc_dbg" if dbg else "nc"
    if key not in _CACHE:
        _CACHE[key] = _build_nc(dbg)
    return _CACHE[key]


def make_in_maps(low, high, q_w, q_b, k_w, k_b, v_w, v_b, o_w, o_b, ln_g, ln_b):
    low_r = np.asarray(low, np.float32).reshape(B, C, N)
    high_r = np.asarray(high, np.float32).reshape(B, C, N)
    f32 = lambda x: np.ascontiguousarray(np.asarray(x, np.float32))
    shared = {
        "wq": f32(np.asarray(q_w, np.float32).T),
        "wk": f32(np.asarray(k_w, np.float32).T),
        "wv": f32(np.asarray(v_w, np.float32).T),
        "wo": f32(np.asarray(o_w, np.float32).T),
        "qb": f32(q_b), "kb": f32(k_b), "vb": f32(v_b), "ob": f32(o_b),
        "lng": f32(ln_g), "lnb": f32(ln_b),
    }
    in_maps = []
    for i in range(8):
        bidx, h = i // 2, i % 2
        in_maps.append({
            "low": f32(low_r[bidx][:, h * NQ:(h + 1) * NQ]),
            "high": f32(high_r[bidx]),
            **shared,
        })
    return in_maps


def assemble(results):
    out = np.empty((B, C, N), np.float32)
    for i in range(8):
        bidx, h = i // 2, i % 2
        out[bidx][:, h * NQ:(h + 1) * NQ] = results[i]["out"]
    return out.reshape(B, C, 64, 64)


def kernel(**inputs) -> np.ndarray:
    nc = get_nc()
    in_maps = make_in_maps(**inputs)
    res = run_bass_kernel_spmd(nc, in_maps, core_ids=list(range(8)))
    return assemble(res.results)


if __name__ == "__main__":
    pass


# revision 24
# speedup vs baseline: 1.3340x; 1.1869x over previous
"""ContentGuidedAttention Trainium2 kernel.

Full NxN single-head cross-attention + out-proj + residual + LayerNorm,
for B=4, C=256, H=W=64 (N=4096 tokens), distributed over 8 NeuronCores:
core i handles batch i//2, query-half i%2 (2048 queries, all 4096 keys).
No collectives: K/V are computed redundantly on the two cores sharing a
batch (~5% extra FLOPs).

Layout strategy (all channel-major, zero transposes):
  - Q^T, K^T computed as [C, n] (channels on partitions) in bf16
  - V computed token-major [n, C] in bf16
  - S^T = K Q^T computed as [k, q] psum tiles; exp on ACT -> P^T bf16
  - softmax denominator: contiguous DVE chunk-tree then a ones-vector
    matmul reduces the 128 partitions -> [1, q]
  - reciprocals and rsqrt run on ACT as exp(-ln x) / exp(-0.5 ln x):
    Ln and Exp share one activation-table set, so no table switches
  - row -> all-partition replication via K=1 ones-column matmuls
  - PV: O^T[c, q] = sum_k V[k,c] P^T[k,q]; out-proj keeps channel-major
  - LN entirely per-query-block, overlapped with the next block's
    attention; affine via ACT Identity with per-partition scale/bias
Projection matmuls run in float32r (full PE rate at free dim >= 256).
"""

import numpy as np

import concourse.bass as bass
import concourse.mybir as mybir
import concourse.tile as tile
from concourse import bacc
from concourse.bass import ds, ts
from concourse.bass_utils import run_bass_kernel_spmd

F32 = mybir.dt.float32
F32R = mybir.dt.float32r
BF16 = mybir.dt.bfloat16
AF = mybir.ActivationFunctionType
OP = mybir.AluOpType

B = 4
C = 256
N = 4096          # tokens per batch
NQ = 2048         # queries per core
QB = 512          # query block
NQB = NQ // QB    # 4
NKC = N // 128    # 32 key chunks
NKR = 4           # key ranges (1024 keys each) for K^T / V tiles
SCALE = (C // 8) ** -0.5
LN_EPS = 1e-5

_CACHE = {}


def _build_nc(dbg=False):
    nc = bacc.Bacc("TRN2", target_bir_lowering=False, debug=False)

    low_d = nc.declare_dram_parameter("low", [C, NQ], F32R, isOutput=False)
    high_d = nc.declare_dram_parameter("high", [C, N], F32R, isOutput=False)
    # weights are passed pre-transposed: [c_in, c_out]
    wq_d = nc.declare_dram_parameter("wq", [C, C], F32R, isOutput=False)
    wk_d = nc.declare_dram_parameter("wk", [C, C], F32R, isOutput=False)
    wv_d = nc.declare_dram_parameter("wv", [C, C], F32R, isOutput=False)
    wo_d = nc.declare_dram_parameter("wo", [C, C], F32R, isOutput=False)
    qb_d = nc.declare_dram_parameter("qb", [C], F32, isOutput=False)
    kb_d = nc.declare_dram_parameter("kb", [C], F32, isOutput=False)
    vb_d = nc.declare_dram_parameter("vb", [C], F32R, isOutput=False)
    ob_d = nc.declare_dram_parameter("ob", [C], F32, isOutput=False)
    lng_d = nc.declare_dram_parameter("lng", [C], F32, isOutput=False)
    lnb_d = nc.declare_dram_parameter("lnb", [C], F32, isOutput=False)
    out_d = nc.declare_dram_parameter("out", [C, NQ], F32, isOutput=True)
    dbg_d = {}
    if dbg:
        for nm, shp, dt_ in [
            ("dbg_rcp", [1, 512], F32), ("dbg_mu", [1, 512], F32),
            ("dbg_var", [1, 512], F32), ("dbg_rstd", [1, 512], F32),
            ("dbg_tT", [128, 512], BF16), ("dbg_ot", [128, 2, QB], F32),
            ("dbg_qt", [128, 2, QB], BF16), ("dbg_kt", [128, 2, 1024], BF16),
            ("dbg_v", [128, 8, C], BF16), ("dbg_pt", [128, 8, QB], BF16),
        ]:
            dbg_d[nm] = nc.declare_dram_parameter(nm, shp, dt_, isOutput=True)

    with tile.TileContext(nc) as tc:
        with (
            tc.tile_pool(name="persist", bufs=1) as pp,
            tc.tile_pool(name="high", bufs=3) as high_pool,
            tc.tile_pool(name="pt", bufs=5) as pt_pool,
            tc.tile_pool(name="ot", bufs=2) as ot_pool,
            tc.tile_pool(name="scratch", bufs=2) as scr_pool,
            tc.tile_pool(name="rowscr", bufs=1) as row_pool,
            tc.tile_pool(name="outsb", bufs=2) as out_pool,
            tc.tile_pool(name="st_ps", bufs=2, space="PSUM") as st_ps,
            tc.tile_pool(name="acc_ps", bufs=3, space="PSUM") as acc_ps,
            tc.tile_pool(name="row_ps", bufs=1, space="PSUM") as row_ps,
        ):
            # ---------------- constants / parameters ----------------
            # one tile per weight matrix ([cin_p, cin_chunk, cout]); K/V
            # weights load first so the K/V projections start ASAP
            wk_sb = pp.tile([128, 2, C], F32R)
            wv_sb = pp.tile([128, 2, C], F32R)
            wq_sb = pp.tile([128, 2, C], F32R)
            wo_sb = pp.tile([128, 2, C], F32R)
            for t, d in [(wk_sb, wk_d), (wv_sb, wv_d), (wq_sb, wq_d),
                         (wo_sb, wo_d)]:
                for j in range(2):
                    nc.sync.dma_start(out=t[:, j, :], in_=d[ds(j * 128, 128), :])

            # v bias as a [1, 256] row (free-dim vector)
            vbrow = pp.tile([1, C], F32R)
            nc.gpsimd.dma_start(out=vbrow[:, :], in_=vb_d[:].unsqueeze(0))

            # memset cannot emit float32r; stage in f32 and copy (the
            # DVE tensor_copy performs the f32 -> f32r rounding walrus wants)
            stage = pp.tile([128, 128], F32)
            ones1 = pp.tile([1, 128], F32R)      # K=1 replication lhsT
            nc.vector.memset(stage[ds(0, 1), :], 1.0)
            nc.vector.tensor_copy(ones1[:, :], stage[ds(0, 1), :])
            ones128 = pp.tile([128, 1], F32R)    # partition-reduce lhsT (f32r)
            nc.vector.memset(stage[:, 0:1], 1.0)
            nc.vector.tensor_copy(ones128[:, :], stage[:, 0:1])
            ones128b = pp.tile([128, 1], BF16)   # partition-reduce lhsT (bf16)
            nc.vector.memset(ones128b[:, :], 1.0)
            epsb = pp.tile([1, 1], F32)          # LN epsilon bias
            nc.vector.memset(epsb[:, :], LN_EPS)

            # per-partition scalars: qb, kb, ob, lng, lnb as [128, 2] each
            pvec = pp.tile([128, 10], F32)
            for vi, vd in enumerate([qb_d, kb_d, ob_d, lng_d, lnb_d]):
                nc.gpsimd.dma_start(
                    out=pvec[:, ds(vi * 2, 2)],
                    in_=vd[:].rearrange("(j p) -> p j", p=128),
                )
            QBIAS, KBIAS, OBIAS, LNG, LNB = 0, 2, 4, 6, 8

            # replicate v bias across partitions: vb_rep[m, c] = vb[c]
            vb_ps = acc_ps.tile([128, C], F32, tag="acc")
            nc.tensor.matmul(
                out=vb_ps[:, :], lhsT=ones1[:, :], rhs=vbrow[:, :],
                start=True, stop=True,
            )
            vb_rep = pp.tile([128, C], F32)
            nc.vector.tensor_copy(vb_rep[:, :], vb_ps[:, :])

            # ---------------- K^T / V projections ----------------
            # per 1024-key-range tiles so attention can start early
            kt_sb = [
                pp.tile([128, 2, 1024], BF16, name=f"kt{r}", tag=f"kt{r}")
                for r in range(NKR)
            ]
            v_sb = [
                pp.tile([128, 8, C], BF16, name=f"v{r}", tag=f"v{r}")
                for r in range(NKR)
            ]
            for kr in range(N // 512):
                hi = high_pool.tile([128, 2, 512], F32R)
                for j in range(2):
                    nc.sync.dma_start(
                        out=hi[:, j, :],
                        in_=high_d[ds(j * 128, 128), ds(kr * 512, 512)],
                    )
                r, h = kr // 2, kr % 2
                # K^T: out [cout, k] = sum_cin wk[cin, cout] high[cin, k]
                for c in range(2):
                    kps = st_ps.tile([128, 512], F32, tag="st")
                    for j in range(2):
                        nc.tensor.matmul(
                            out=kps[:, :],
                            lhsT=wk_sb[:, j, ds(c * 128, 128)],
                            rhs=hi[:, j, :],
                            start=(j == 0), stop=(j == 1),
                        )
                    nc.vector.tensor_scalar_add(
                        out=kt_sb[r][:, c, ds(h * 512, 512)],
                        in0=kps[:, :],
                        scalar1=pvec[:, ds(KBIAS + c, 1)],
                    )
                # V: out [k, cout] = sum_cin high[cin, k] wv[cin, cout]
                for u in range(4):
                    vps = st_ps.tile([128, C], F32, tag="st")
                    for j in range(2):
                        nc.tensor.matmul(
                            out=vps[:, :],
                            lhsT=hi[:, j, ds(u * 128, 128)],
                            rhs=wv_sb[:, j, :],
                            start=(j == 0), stop=(j == 1),
                        )
                    nc.vector.tensor_add(
                        out=v_sb[r][:, h * 4 + u, :],
                        in0=vps[:, :], in1=vb_rep[:, :],
                    )

            # ---------------- Q^T projection (all blocks) ----------------
            low_sb = pp.tile([128, 2, NQ], F32R)
            for j in range(2):
                nc.scalar.dma_start(
                    out=low_sb[:, j, :], in_=low_d[ds(j * 128, 128), :]
                )
            qt_all = pp.tile([128, 2, NQ], BF16)
            for qb4 in range(NQB):
                for c in range(2):
                    qps = st_ps.tile([128, QB], F32, tag="st")
                    for j in range(2):
                        nc.tensor.matmul(
                            out=qps[:, :],
                            lhsT=wq_sb[:, j, ds(c * 128, 128)],
                            rhs=low_sb[:, j, ds(qb4 * QB, QB)],
                            start=(j == 0), stop=(j == 1),
                        )
                    nc.vector.tensor_scalar_add(
                        out=qt_all[:, c, ds(qb4 * QB, QB)], in0=qps[:, :],
                        scalar1=pvec[:, ds(QBIAS + c, 1)],
                    )

            # ---------------- main loop over query blocks ----------------
            for b in range(NQB):
                qsl = ds(b * QB, QB)

                # ---- S^T + exp, in pairs of key chunks ----
                quarters = [
                    pt_pool.tile([128, 8, QB], BF16, tag="ptq", name=f"ptq{g}")
                    for g in range(4)
                ]
                for si in range(16):
                    sps = st_ps.tile([128, 2, QB], F32, tag="st")
                    for u in range(2):
                        kc = si * 2 + u
                        for c in range(2):
                            nc.tensor.matmul(
                                out=sps[:, u, :],
                                lhsT=kt_sb[kc // 8][:, c, ds((kc % 8) * 128, 128)],
                                rhs=qt_all[:, c, qsl],
                                start=(c == 0), stop=(c == 1),
                            )
                    nc.scalar.activation(
                        out=quarters[si // 4][:, ds((si % 4) * 2, 2), :],
                        in_=sps[:, :, :],
                        func=AF.Exp,
                        scale=SCALE,
                    )

                # ---- PV: O^T[c, q] accumulated over 32 key chunks ----
                ot = ot_pool.tile([128, 2, QB], F32R)
                for c in range(2):
                    ops = acc_ps.tile([128, QB], F32, tag="acc")
                    for kc in range(NKC):
                        nc.tensor.matmul(
                            out=ops[:, :],
                            lhsT=v_sb[kc // 8][:, kc % 8, ds(c * 128, 128)],
                            rhs=quarters[kc // 8][:, kc % 8, :],
                            start=(kc == 0), stop=(kc == NKC - 1),
                        )
                    nc.vector.tensor_copy(ot[:, c, :], ops[:, :])

                # ---- softmax denominator: contiguous DVE chunk tree ----
                fl = [q[:, :, :].rearrange("p a b -> p (a b)") for q in quarters]
                nc.vector.tensor_add(out=fl[0], in0=fl[0], in1=fl[1])
                nc.vector.tensor_add(out=fl[2], in0=fl[2], in1=fl[3])
                nc.vector.tensor_add(out=fl[0], in0=fl[0], in1=fl[2])
                tB = scr_pool.tile([128, 4, QB], BF16, tag="tB")
                nc.vector.tensor_add(
                    out=tB[:, :, :],
                    in0=quarters[0][:, 0:4, :], in1=quarters[0][:, 4:8, :],
                )
                tT2 = scr_pool.tile([128, 2, QB], BF16, tag="tT2")
                nc.vector.tensor_add(
                    out=tT2[:, :, :], in0=tB[:, 0:2, :], in1=tB[:, 2:4, :]
                )
                tT = scr_pool.tile([128, QB], BF16, tag="tT")
                nc.vector.tensor_add(
                    out=tT[:, :], in0=tT2[:, 0, :], in1=tT2[:, 1, :]
                )
                dn_ps = row_ps.tile([1, QB], F32, tag="row")
                nc.tensor.matmul(
                    out=dn_ps[:, :], lhsT=ones128b[:, :], rhs=tT[:, :],
                    start=True, stop=True,
                )
                # 1/denom = exp(-ln(denom)) on ACT (same table set as Exp)
                lnrow = row_pool.tile([1, QB], F32, tag="lnrow")
                nc.scalar.activation(
                    out=lnrow[:, :], in_=dn_ps[:, :], func=AF.Ln
                )
                rcprow = row_pool.tile([1, QB], F32, tag="rcprow")
                nc.scalar.activation(
                    out=rcprow[:, :], in_=lnrow[:, :], func=AF.Exp, scale=-1.0
                )
                rcp_rep = scr_pool.tile([128, QB], F32, tag="rcprep")
                nc.gpsimd.partition_broadcast(rcp_rep[:, :], rcprow[:, :])

                # ---- out-projection + 1/denom + residual + out-bias ----
                y_sb = ot_pool.tile([128, 2, QB], F32R, tag="y")
                for c in range(2):
                    pps = acc_ps.tile([128, QB], F32, tag="acc")
                    for j in range(2):
                        nc.tensor.matmul(
                            out=pps[:, :],
                            lhsT=wo_sb[:, j, ds(c * 128, 128)],
                            rhs=ot[:, j, :],
                            start=(j == 0), stop=(j == 1),
                        )
                    ysc = scr_pool.tile([128, QB], F32, tag="scr")
                    nc.vector.tensor_mul(
                        out=ysc[:, :], in0=pps[:, :], in1=rcp_rep[:, :]
                    )
                    nc.vector.scalar_tensor_tensor(
                        out=y_sb[:, c, :],
                        in0=ysc[:, :],
                        scalar=pvec[:, ds(OBIAS + c, 1)],
                        in1=low_sb[:, c, qsl].bitcast(F32),
                        op0=OP.add, op1=OP.add,
                    )

                # ---- LN statistics (rows at partition 0) ----
                sy_ps = row_ps.tile([1, QB], F32, tag="row")
                for c in range(2):
                    nc.tensor.matmul(
                        out=sy_ps[:, :],
                        lhsT=ones128[:, :],
                        rhs=y_sb[:, c, :],
                        start=(c == 0), stop=(c == 1),
                    )
                murow = row_pool.tile([1, QB], F32, tag="murow")
                nc.vector.tensor_scalar_mul(
                    out=murow[:, :], in0=sy_ps[:, :], scalar1=1.0 / C
                )
                sy2_ps = row_ps.tile([1, QB], F32, tag="row")
                for c in range(2):
                    ysq = scr_pool.tile([128, QB], F32R, tag="ysq")
                    nc.vector.tensor_mul(
                        out=ysq[:, :],
                        in0=y_sb[:, c, :].bitcast(F32),
                        in1=y_sb[:, c, :].bitcast(F32),
                    )
                    nc.tensor.matmul(
                        out=sy2_ps[:, :],
                        lhsT=ones128[:, :],
                        rhs=ysq[:, :],
                        start=(c == 0), stop=(c == 1),
                    )
                # var = E[y^2] - mu^2 ; rstd = exp(-0.5 ln(var + eps))
                varrow = row_pool.tile([1, QB], F32, tag="varrow")
                nc.vector.tensor_scalar_mul(
                    out=varrow[:, :], in0=sy2_ps[:, :], scalar1=1.0 / C
                )
                mu2row = row_pool.tile([1, QB], F32, tag="mu2row")
                nc.vector.tensor_mul(
                    out=mu2row[:, :],
                    in0=murow[:, :], in1=murow[:, :],
                )
                nc.vector.tensor_sub(
                    out=varrow[:, :], in0=varrow[:, :], in1=mu2row[:, :]
                )
                lnv = row_pool.tile([1, QB], F32, tag="lnv")
                nc.scalar.activation(
                    out=lnv[:, :], in_=varrow[:, :], func=AF.Ln, bias=epsb[:, :]
                )
                rstdrow = row_pool.tile([1, QB], F32, tag="rstdrow")
                nc.scalar.activation(
                    out=rstdrow[:, :], in_=lnv[:, :], func=AF.Exp, scale=-0.5
                )
                if dbg_d and b == NQB - 1:
                    nc.sync.dma_start(out=dbg_d["dbg_rcp"][:, :], in_=rcprow[:, :])
                    nc.sync.dma_start(out=dbg_d["dbg_mu"][:, :], in_=murow[:, :])
                    nc.sync.dma_start(out=dbg_d["dbg_var"][:, :],
                                      in_=varrow[:, :])
                    nc.sync.dma_start(out=dbg_d["dbg_rstd"][:, :], in_=rstdrow[:, :])

                # ---- replicate mu / rstd, normalize, affine, store ----
                mu_rep = scr_pool.tile([128, QB], F32, tag="murep")
                nc.gpsimd.partition_broadcast(mu_rep[:, :], murow[:, :])
                rs_rep = scr_pool.tile([128, QB], F32, tag="rsrep")
                nc.gpsimd.partition_broadcast(rs_rep[:, :], rstdrow[:, :])
                for c in range(2):
                    yn = scr_pool.tile([128, QB], F32, tag="scr")
                    nc.vector.tensor_sub(
                        out=yn[:, :],
                        in0=y_sb[:, c, :].bitcast(F32),
                        in1=mu_rep[:, :],
                    )
                    nc.vector.tensor_mul(
                        out=yn[:, :], in0=yn[:, :], in1=rs_rep[:, :]
                    )
                    osb = out_pool.tile([128, QB], F32)
                    nc.vector.tensor_scalar(
                        out=osb[:, :], in0=yn[:, :],
                        scalar1=pvec[:, ds(LNG + c, 1)],
                        scalar2=pvec[:, ds(LNB + c, 1)],
                        op0=OP.mult, op1=OP.add,
                    )
                    nc.scalar.dma_start(
                        out=out_d[ds(c * 128, 128), qsl], in_=osb[:, :]
                    )

                if dbg_d and b == NQB - 1:
                    nc.sync.dma_start(out=dbg_d["dbg_tT"][:, :], in_=tT[:, :])
                    nc.sync.dma_start(
                        out=dbg_d["dbg_ot"][:, :, :], in_=ot[:, :, :].bitcast(F32)
                    )
                    nc.sync.dma_start(out=dbg_d["dbg_qt"][:, :, :],
                                      in_=qt_all[:, :, 3 * QB:4 * QB])
                    nc.sync.dma_start(
                        out=dbg_d["dbg_kt"][:, :, :], in_=kt_sb[0][:, :, :]
                    )
                    nc.sync.dma_start(
                        out=dbg_d["dbg_v"][:, :, :], in_=v_sb[0][:, :, :]
                    )
                    nc.sync.dma_start(
                        out=dbg_d["dbg_pt"][:, :, :], in_=quarters[3][:, :, :]
                    )

    # Force Exp and Ln to resolve to the one table set containing both
    # (the default chooser alternates exp_and_others <-> natural_log_exp,
    # paying a ~1.3us table load per switch, ~17 loads per kernel).
    import bass_rust as _br
    from concourse.hw_specs import get_activation_tables as _gat

    def _patched_act_loads():
        has_act = any(
            isinstance(i, mybir.InstActivation)
            for blk in nc.main_func.blocks for i in blk.instructions
        )
        if not has_act:
            return
        tables = []
        for name, fns in _gat(nc.m.arch).items():
            if name != "natural_log_exp_and_others":
                fns = fns - {AF.Exp, AF.Ln}
            tables.append((name, fns))
        _br.insert_act_table_loads(nc, tables)

    nc.insert_act_table_loads = _patched_act_loads
    nc.compile()
    return nc


def get_nc(dbg=False):
    key = "nc_dbg" if dbg else "nc"
    if key not in _CACHE:
        _CACHE[key] = _build_nc(dbg)
    return _CACHE[key]


def make_in_maps(low, high, q_w, q_b, k_w, k_b, v_w, v_b, o_w, o_b, ln_g, ln_b):
    low_r = np.asarray(low, np.float32).reshape(B, C, N)
    high_r = np.asarray(high, np.float32).reshape(B, C, N)
    f32 = lambda x: np.ascontiguousarray(np.asarray(x, np.float32))
    shared = {
        "wq": f32(np.asarray(q_w, np.float32).T),
        "wk": f32(np.asarray(k_w, np.float32).T),
        "wv": f32(np.asarray(v_w, np.float32).T),
        "wo": f32(np.asarray(o_w, np.float32).T),
        "qb": f32(q_b), "kb": f32(k_b), "vb": f32(v_b), "ob": f32(o_b),
        "lng": f32(ln_g), "lnb": f32(ln_b),
    }
    in_maps = []
    for i in range(8):
        bidx, h = i // 2, i % 2
        in_maps.append({
            "low": f32(low_r[bidx][:, h * NQ:(h + 1) * NQ]),
            "high": f32(high_r[bidx]),
            **shared,
        })
    return in_maps


def assemble(results):
    out = np.empty((B, C, N), np.float32)
    for i in range(8):
        bidx, h = i // 2, i % 2
        out[bidx][:, h * NQ:(h + 1) * NQ] = results[i]["out"]
    return out.reshape(B, C, 64, 64)


def kernel(**inputs) -> np.ndarray:
    nc = get_nc()
    in_maps = make_in_maps(**inputs)
    res = run_bass_kernel_spmd(nc, in_maps, core_ids=list(range(8)))
    return assemble(res.results)


if __name__ == "__main__":
    pass


# revision 25
# speedup vs baseline: 1.3761x; 1.0315x over previous
"""ContentGuidedAttention Trainium2 kernel.

Full NxN single-head cross-attention + out-proj + residual + LayerNorm,
for B=4, C=256, H=W=64 (N=4096 tokens), distributed over 8 NeuronCores:
core i handles batch i//2, query-half i%2 (2048 queries, all 4096 keys).
No collectives: K/V are computed redundantly on the two cores sharing a
batch (~5% extra FLOPs).

Layout strategy (all channel-major, zero transposes):
  - Q^T, K^T computed as [C, n] (channels on partitions) in bf16
  - V computed token-major [n, C] in bf16
  - S^T = K Q^T computed as [k, q] psum tiles; exp on ACT -> P^T bf16
  - softmax denominator: contiguous DVE chunk-tree then a ones-vector
    matmul reduces the 128 partitions -> [1, q]
  - reciprocals and rsqrt run on ACT as exp(-ln x) / exp(-0.5 ln x):
    Ln and Exp share one activation-table set, so no table switches
  - row -> all-partition replication via K=1 ones-column matmuls
  - PV: O^T[c, q] = sum_k V[k,c] P^T[k,q]; out-proj keeps channel-major
  - LN entirely per-query-block, overlapped with the next block's
    attention; affine via ACT Identity with per-partition scale/bias
Projection matmuls run in float32r (full PE rate at free dim >= 256).
"""

import numpy as np

import concourse.bass as bass
import concourse.mybir as mybir
import concourse.tile as tile
from concourse import bacc
from concourse.bass import ds, ts
from concourse.bass_utils import run_bass_kernel_spmd

F32 = mybir.dt.float32
F32R = mybir.dt.float32r
BF16 = mybir.dt.bfloat16
AF = mybir.ActivationFunctionType
OP = mybir.AluOpType

B = 4
C = 256
N = 4096          # tokens per batch
NQ = 2048         # queries per core
QB = 512          # query block
NQB = NQ // QB    # 4
NKC = N // 128    # 32 key chunks
NKR = 4           # key ranges (1024 keys each) for K^T / V tiles
SCALE = (C // 8) ** -0.5
LN_EPS = 1e-5

_CACHE = {}


def _build_nc(dbg=False):
    nc = bacc.Bacc("TRN2", target_bir_lowering=False, debug=False)

    low_d = nc.declare_dram_parameter("low", [C, NQ], F32R, isOutput=False)
    high_d = nc.declare_dram_parameter("high", [C, N], F32R, isOutput=False)
    # weights are passed pre-transposed: [c_in, c_out]
    wq_d = nc.declare_dram_parameter("wq", [C, C], F32R, isOutput=False)
    wk_d = nc.declare_dram_parameter("wk", [C, C], F32R, isOutput=False)
    wv_d = nc.declare_dram_parameter("wv", [C, C], F32R, isOutput=False)
    wo_d = nc.declare_dram_parameter("wo", [C, C], F32R, isOutput=False)
    qb_d = nc.declare_dram_parameter("qb", [C], F32, isOutput=False)
    kb_d = nc.declare_dram_parameter("kb", [C], F32, isOutput=False)
    vb_d = nc.declare_dram_parameter("vb", [C], F32R, isOutput=False)
    ob_d = nc.declare_dram_parameter("ob", [C], F32, isOutput=False)
    lng_d = nc.declare_dram_parameter("lng", [C], F32, isOutput=False)
    lnb_d = nc.declare_dram_parameter("lnb", [C], F32, isOutput=False)
    out_d = nc.declare_dram_parameter("out", [C, NQ], F32, isOutput=True)
    dbg_d = {}
    if dbg:
        for nm, shp, dt_ in [
            ("dbg_rcp", [1, 512], F32), ("dbg_mu", [1, 512], F32),
            ("dbg_var", [1, 512], F32), ("dbg_rstd", [1, 512], F32),
            ("dbg_tT", [128, 512], BF16), ("dbg_ot", [128, 2, QB], F32),
            ("dbg_qt", [128, 2, QB], BF16), ("dbg_kt", [128, 2, 1024], BF16),
            ("dbg_v", [128, 8, C], BF16), ("dbg_pt", [128, 8, QB], BF16),
        ]:
            dbg_d[nm] = nc.declare_dram_parameter(nm, shp, dt_, isOutput=True)

    with tile.TileContext(nc) as tc:
        with (
            tc.tile_pool(name="persist", bufs=1) as pp,
            tc.tile_pool(name="high", bufs=3) as high_pool,
            tc.tile_pool(name="pt", bufs=5) as pt_pool,
            tc.tile_pool(name="ot", bufs=2) as ot_pool,
            tc.tile_pool(name="scratch", bufs=2) as scr_pool,
            tc.tile_pool(name="rowscr", bufs=1) as row_pool,
            tc.tile_pool(name="outsb", bufs=2) as out_pool,
            tc.tile_pool(name="st_ps", bufs=2, space="PSUM") as st_ps,
            tc.tile_pool(name="acc_ps", bufs=3, space="PSUM") as acc_ps,
            tc.tile_pool(name="row_ps", bufs=1, space="PSUM") as row_ps,
        ):
            # ---------------- constants / parameters ----------------
            # one tile per weight matrix ([cin_p, cin_chunk, cout]); K/V
            # weights load first so the K/V projections start ASAP
            wk_sb = pp.tile([128, 2, C], F32R)
            wv_sb = pp.tile([128, 2, C], F32R)
            wq_sb = pp.tile([128, 2, C], F32R)
            wo_sb = pp.tile([128, 2, C], F32R)
            for t, d in [(wk_sb, wk_d), (wv_sb, wv_d), (wq_sb, wq_d),
                         (wo_sb, wo_d)]:
                for j in range(2):
                    nc.scalar.dma_start(out=t[:, j, :], in_=d[ds(j * 128, 128), :])

            # v bias as a [1, 256] row (free-dim vector)
            vbrow = pp.tile([1, C], F32R)
            nc.gpsimd.dma_start(out=vbrow[:, :], in_=vb_d[:].unsqueeze(0))

            # memset cannot emit float32r; stage in f32 and copy (the
            # DVE tensor_copy performs the f32 -> f32r rounding walrus wants)
            stage = pp.tile([128, 128], F32)
            ones1 = pp.tile([1, 128], F32R)      # K=1 replication lhsT
            nc.vector.memset(stage[ds(0, 1), :], 1.0)
            nc.vector.tensor_copy(ones1[:, :], stage[ds(0, 1), :])
            ones128 = pp.tile([128, 1], F32R)    # partition-reduce lhsT (f32r)
            nc.vector.memset(stage[:, 0:1], 1.0)
            nc.vector.tensor_copy(ones128[:, :], stage[:, 0:1])
            ones128b = pp.tile([128, 1], BF16)   # partition-reduce lhsT (bf16)
            nc.vector.memset(ones128b[:, :], 1.0)
            epsb = pp.tile([1, 1], F32)          # LN epsilon bias
            nc.vector.memset(epsb[:, :], LN_EPS)

            # per-partition scalars: qb, kb, ob, lng, lnb as [128, 2] each
            pvec = pp.tile([128, 10], F32)
            for vi, vd in enumerate([qb_d, kb_d, ob_d, lng_d, lnb_d]):
                nc.gpsimd.dma_start(
                    out=pvec[:, ds(vi * 2, 2)],
                    in_=vd[:].rearrange("(j p) -> p j", p=128),
                )
            QBIAS, KBIAS, OBIAS, LNG, LNB = 0, 2, 4, 6, 8

            # ---------------- K^T / V projections ----------------
            # per 1024-key-range tiles so attention can start early
            kt_sb = [
                pp.tile([128, 2, 1024], BF16, name=f"kt{r}", tag=f"kt{r}")
                for r in range(NKR)
            ]
            v_sb = [
                pp.tile([128, 8, C], BF16, name=f"v{r}", tag=f"v{r}")
                for r in range(NKR)
            ]
            for kr in range(N // 512):
                hi = high_pool.tile([128, 2, 512], F32R)
                for j in range(2):
                    nc.sync.dma_start(
                        out=hi[:, j, :],
                        in_=high_d[ds(j * 128, 128), ds(kr * 512, 512)],
                    )
                r, h = kr // 2, kr % 2
                # K^T: out [cout, k] = sum_cin wk[cin, cout] high[cin, k]
                for c in range(2):
                    kps = st_ps.tile([128, 512], F32, tag="st")
                    for j in range(2):
                        nc.tensor.matmul(
                            out=kps[:, :],
                            lhsT=wk_sb[:, j, ds(c * 128, 128)],
                            rhs=hi[:, j, :],
                            start=(j == 0), stop=(j == 1),
                        )
                    nc.vector.tensor_scalar_add(
                        out=kt_sb[r][:, c, ds(h * 512, 512)],
                        in0=kps[:, :],
                        scalar1=pvec[:, ds(KBIAS + c, 1)],
                    )
                if kr == 0:
                    # v-bias replication, placed here so its (slow) DMA +
                    # memset dependencies never stall the in-order PE queue
                    vb_ps = acc_ps.tile([128, C], F32, tag="acc")
                    nc.tensor.matmul(
                        out=vb_ps[:, :], lhsT=ones1[:, :], rhs=vbrow[:, :],
                        start=True, stop=True,
                    )
                    vb_rep = pp.tile([128, C], F32)
                    nc.vector.tensor_copy(vb_rep[:, :], vb_ps[:, :])
                # V: out [k, cout] = sum_cin high[cin, k] wv[cin, cout]
                for u in range(4):
                    vps = st_ps.tile([128, C], F32, tag="st")
                    for j in range(2):
                        nc.tensor.matmul(
                            out=vps[:, :],
                            lhsT=hi[:, j, ds(u * 128, 128)],
                            rhs=wv_sb[:, j, :],
                            start=(j == 0), stop=(j == 1),
                        )
                    nc.vector.tensor_add(
                        out=v_sb[r][:, h * 4 + u, :],
                        in0=vps[:, :], in1=vb_rep[:, :],
                    )

            # ---------------- Q^T projection (all blocks) ----------------
            low_sb = pp.tile([128, 2, NQ], F32R)
            for j in range(2):
                nc.scalar.dma_start(
                    out=low_sb[:, j, :], in_=low_d[ds(j * 128, 128), :]
                )
            qt_all = pp.tile([128, 2, NQ], BF16)
            for qb4 in range(NQB):
                for c in range(2):
                    qps = st_ps.tile([128, QB], F32, tag="st")
                    for j in range(2):
                        nc.tensor.matmul(
                            out=qps[:, :],
                            lhsT=wq_sb[:, j, ds(c * 128, 128)],
                            rhs=low_sb[:, j, ds(qb4 * QB, QB)],
                            start=(j == 0), stop=(j == 1),
                        )
                    nc.vector.tensor_scalar_add(
                        out=qt_all[:, c, ds(qb4 * QB, QB)], in0=qps[:, :],
                        scalar1=pvec[:, ds(QBIAS + c, 1)],
                    )

            # ---------------- main loop over query blocks ----------------
            for b in range(NQB):
                qsl = ds(b * QB, QB)

                # ---- S^T + exp, in pairs of key chunks ----
                quarters = [
                    pt_pool.tile([128, 8, QB], BF16, tag="ptq", name=f"ptq{g}")
                    for g in range(4)
                ]
                for si in range(16):
                    sps = st_ps.tile([128, 2, QB], F32, tag="st")
                    for u in range(2):
                        kc = si * 2 + u
                        for c in range(2):
                            nc.tensor.matmul(
                                out=sps[:, u, :],
                                lhsT=kt_sb[kc // 8][:, c, ds((kc % 8) * 128, 128)],
                                rhs=qt_all[:, c, qsl],
                                start=(c == 0), stop=(c == 1),
                            )
                    nc.scalar.activation(
                        out=quarters[si // 4][:, ds((si % 4) * 2, 2), :],
                        in_=sps[:, :, :],
                        func=AF.Exp,
                        scale=SCALE,
                    )

                # ---- PV: O^T[c, q] accumulated over 32 key chunks ----
                ot = ot_pool.tile([128, 2, QB], F32R)
                for c in range(2):
                    ops = acc_ps.tile([128, QB], F32, tag="acc")
                    for kc in range(NKC):
                        nc.tensor.matmul(
                            out=ops[:, :],
                            lhsT=v_sb[kc // 8][:, kc % 8, ds(c * 128, 128)],
                            rhs=quarters[kc // 8][:, kc % 8, :],
                            start=(kc == 0), stop=(kc == NKC - 1),
                        )
                    nc.vector.tensor_copy(ot[:, c, :], ops[:, :])

                # ---- softmax denominator: contiguous DVE chunk tree ----
                fl = [q[:, :, :].rearrange("p a b -> p (a b)") for q in quarters]
                nc.vector.tensor_add(out=fl[0], in0=fl[0], in1=fl[1])
                nc.vector.tensor_add(out=fl[2], in0=fl[2], in1=fl[3])
                nc.vector.tensor_add(out=fl[0], in0=fl[0], in1=fl[2])
                tB = scr_pool.tile([128, 4, QB], BF16, tag="tB")
                nc.vector.tensor_add(
                    out=tB[:, :, :],
                    in0=quarters[0][:, 0:4, :], in1=quarters[0][:, 4:8, :],
                )
                tT2 = scr_pool.tile([128, 2, QB], BF16, tag="tT2")
                nc.vector.tensor_add(
                    out=tT2[:, :, :], in0=tB[:, 0:2, :], in1=tB[:, 2:4, :]
                )
                tT = scr_pool.tile([128, QB], BF16, tag="tT")
                nc.vector.tensor_add(
                    out=tT[:, :], in0=tT2[:, 0, :], in1=tT2[:, 1, :]
                )
                dn_ps = row_ps.tile([1, QB], F32, tag="row")
                nc.tensor.matmul(
                    out=dn_ps[:, :], lhsT=ones128b[:, :], rhs=tT[:, :],
                    start=True, stop=True,
                )
                # 1/denom = exp(-ln(denom)) on ACT (same table set as Exp)
                lnrow = row_pool.tile([1, QB], F32, tag="lnrow")
                nc.scalar.activation(
                    out=lnrow[:, :], in_=dn_ps[:, :], func=AF.Ln
                )
                rcprow = row_pool.tile([1, QB], F32, tag="rcprow")
                nc.scalar.activation(
                    out=rcprow[:, :], in_=lnrow[:, :], func=AF.Exp, scale=-1.0
                )
                rcp_rep = scr_pool.tile([128, QB], F32, tag="rcprep")
                nc.gpsimd.partition_broadcast(rcp_rep[:, :], rcprow[:, :])

                # ---- out-projection + 1/denom + residual + out-bias ----
                y_sb = ot_pool.tile([128, 2, QB], F32R, tag="y")
                for c in range(2):
                    pps = acc_ps.tile([128, QB], F32, tag="acc")
                    for j in range(2):
                        nc.tensor.matmul(
                            out=pps[:, :],
                            lhsT=wo_sb[:, j, ds(c * 128, 128)],
                            rhs=ot[:, j, :],
                            start=(j == 0), stop=(j == 1),
                        )
                    ysc = scr_pool.tile([128, QB], F32, tag="scr")
                    nc.vector.tensor_mul(
                        out=ysc[:, :], in0=pps[:, :], in1=rcp_rep[:, :]
                    )
                    nc.vector.scalar_tensor_tensor(
                        out=y_sb[:, c, :],
                        in0=ysc[:, :],
                        scalar=pvec[:, ds(OBIAS + c, 1)],
                        in1=low_sb[:, c, qsl].bitcast(F32),
                        op0=OP.add, op1=OP.add,
                    )

                # ---- LN statistics (rows at partition 0) ----
                sy_ps = row_ps.tile([1, QB], F32, tag="row")
                for c in range(2):
                    nc.tensor.matmul(
                        out=sy_ps[:, :],
                        lhsT=ones128[:, :],
                        rhs=y_sb[:, c, :],
                        start=(c == 0), stop=(c == 1),
                    )
                murow = row_pool.tile([1, QB], F32, tag="murow")
                nc.vector.tensor_scalar_mul(
                    out=murow[:, :], in0=sy_ps[:, :], scalar1=1.0 / C
                )
                sy2_ps = row_ps.tile([1, QB], F32, tag="row")
                for c in range(2):
                    ysq = scr_pool.tile([128, QB], F32R, tag="ysq")
                    nc.vector.tensor_mul(
                        out=ysq[:, :],
                        in0=y_sb[:, c, :].bitcast(F32),
                        in1=y_sb[:, c, :].bitcast(F32),
                    )
                    nc.tensor.matmul(
                        out=sy2_ps[:, :],
                        lhsT=ones128[:, :],
                        rhs=ysq[:, :],
                        start=(c == 0), stop=(c == 1),
                    )
                # var = E[y^2] - mu^2 ; rstd = exp(-0.5 ln(var + eps))
                varrow = row_pool.tile([1, QB], F32, tag="varrow")
                nc.vector.tensor_scalar_mul(
                    out=varrow[:, :], in0=sy2_ps[:, :], scalar1=1.0 / C
                )
                mu2row = row_pool.tile([1, QB], F32, tag="mu2row")
                nc.vector.tensor_mul(
                    out=mu2row[:, :],
                    in0=murow[:, :], in1=murow[:, :],
                )
                nc.vector.tensor_sub(
                    out=varrow[:, :], in0=varrow[:, :], in1=mu2row[:, :]
                )
                lnv = row_pool.tile([1, QB], F32, tag="lnv")
                nc.scalar.activation(
                    out=lnv[:, :], in_=varrow[:, :], func=AF.Ln, bias=epsb[:, :]
                )
                rstdrow = row_pool.tile([1, QB], F32, tag="rstdrow")
                nc.scalar.activation(
                    out=rstdrow[:, :], in_=lnv[:, :], func=AF.Exp, scale=-0.5
                )
                if dbg_d and b == NQB - 1:
                    nc.sync.dma_start(out=dbg_d["dbg_rcp"][:, :], in_=rcprow[:, :])
                    nc.sync.dma_start(out=dbg_d["dbg_mu"][:, :], in_=murow[:, :])
                    nc.sync.dma_start(out=dbg_d["dbg_var"][:, :],
                                      in_=varrow[:, :])
                    nc.sync.dma_start(out=dbg_d["dbg_rstd"][:, :], in_=rstdrow[:, :])

                # ---- replicate mu / rstd, normalize, affine, store ----
                mu_rep = scr_pool.tile([128, QB], F32, tag="murep")
                nc.gpsimd.partition_broadcast(mu_rep[:, :], murow[:, :])
                rs_rep = scr_pool.tile([128, QB], F32, tag="rsrep")
                nc.gpsimd.partition_broadcast(rs_rep[:, :], rstdrow[:, :])
                for c in range(2):
                    yn = scr_pool.tile([128, QB], F32, tag="scr")
                    nc.vector.tensor_sub(
                        out=yn[:, :],
                        in0=y_sb[:, c, :].bitcast(F32),
                        in1=mu_rep[:, :],
                    )
                    nc.vector.tensor_mul(
                        out=yn[:, :], in0=yn[:, :], in1=rs_rep[:, :]
                    )
                    osb = out_pool.tile([128, QB], F32)
                    nc.vector.tensor_scalar(
                        out=osb[:, :], in0=yn[:, :],
                        scalar1=pvec[:, ds(LNG + c, 1)],
                        scalar2=pvec[:, ds(LNB + c, 1)],
                        op0=OP.mult, op1=OP.add,
                    )
                    nc.scalar.dma_start(
                        out=out_d[ds(c * 128, 128), qsl], in_=osb[:, :]
                    )

                if dbg_d and b == NQB - 1:
                    nc.sync.dma_start(out=dbg_d["dbg_tT"][:, :], in_=tT[:, :])
                    nc.sync.dma_start(
                        out=dbg_d["dbg_ot"][:, :, :], in_=ot[:, :, :].bitcast(F32)
                    )
                    nc.sync.dma_start(out=dbg_d["dbg_qt"][:, :, :],
                                      in_=qt_all[:, :, 3 * QB:4 * QB])
                    nc.sync.dma_start(
                        out=dbg_d["dbg_kt"][:, :, :], in_=kt_sb[0][:, :, :]
                    )
                    nc.sync.dma_start(
                        out=dbg_d["dbg_v"][:, :, :], in_=v_sb[0][:, :, :]
                    )
                    nc.sync.dma_start(
                        out=dbg_d["dbg_pt"][:, :, :], in_=quarters[3][:, :, :]
                    )

    # Force Exp and Ln to resolve to the one table set containing both
    # (the default chooser alternates exp_and_others <-> natural_log_exp,
    # paying a ~1.3us table load per switch, ~17 loads per kernel).
    import bass_rust as _br
    from concourse.hw_specs import get_activation_tables as _gat

    def _patched_act_loads():
        has_act = any(
            isinstance(i, mybir.InstActivation)
            for blk in nc.main_func.blocks for i in blk.instructions
        )
        if not has_act:
            return
        tables = []
        for name, fns in _gat(nc.m.arch).items():
            if name != "natural_log_exp_and_others":
                fns = fns - {AF.Exp, AF.Ln}
            tables.append((name, fns))
        _br.insert_act_table_loads(nc, tables)

    nc.insert_act_table_loads = _patched_act_loads
    nc.compile()
    return nc


def get_nc(dbg=False):
    key = "nc_dbg" if dbg else "nc"
    if key not in _CACHE:
        _CACHE[key] = _build_nc(dbg)
    return _CACHE[key]


def make_in_maps(low, high, q_w, q_b, k_w, k_b, v_w, v_b, o_w, o_b, ln_g, ln_b):
    low_r = np.asarray(low, np.float32).reshape(B, C, N)
    high_r = np.asarray(high, np.float32).reshape(B, C, N)
    f32 = lambda x: np.ascontiguousarray(np.asarray(x, np.float32))
    shared = {
        "wq": f32(np.asarray(q_w, np.float32).T),
        "wk": f32(np.asarray(k_w, np.float32).T),
        "wv": f32(np.asarray(v_w, np.float32).T),
        "wo": f32(np.asarray(o_w, np.float32).T),
        "qb": f32(q_b), "kb": f32(k_b), "vb": f32(v_b), "ob": f32(o_b),
        "lng": f32(ln_g), "lnb": f32(ln_b),
    }
    in_maps = []
    for i in range(8):
        bidx, h = i // 2, i % 2
        in_maps.append({
            "low": f32(low_r[bidx][:, h * NQ:(h + 1) * NQ]),
            "high": f32(high_r[bidx]),
            **shared,
        })
    return in_maps


def assemble(results):
    out = np.empty((B, C, N), np.float32)
    for i in range(8):
        bidx, h = i // 2, i % 2
        out[bidx][:, h * NQ:(h + 1) * NQ] = results[i]["out"]
    return out.reshape(B, C, 64, 64)


def kernel(**inputs) -> np.ndarray:
    nc = get_nc()
    in_maps = make_in_maps(**inputs)
    res = run_bass_kernel_spmd(nc, in_maps, core_ids=list(range(8)))
    return assemble(res.results)


if __name__ == "__main__":
    pass
